# revision 1
# baseline (speedup 1.0000x reference)
"""Multi-head attention (B=8, N=1024, H=12, D=64, C=768) on 8 trn2 cores.

Sharding: data-parallel over batch. Core b computes attention for x[b];
weights are replicated. No collectives.

Per-core dataflow (all matmul operands float32r = full PE rate, fp32 bits):
  phase 1a: qkT[1536 x N] = W_qkv[:, :1536].T @ x^T    (d-major Q^T, K^T)
  phase 1b: v[N x 768]    = x @ W_qkv[:, 1536:]        (+ ones column per head)
  phase 2 (per head pair, heads 2t/2t+1 packed at partitions 0:64/64:128):
     S^T[m,n] = k^T.T @ q^T            (K=64 row-group packed pairs)
     P^T = exp(S^T / 8)                 (ScalarE, one [128,1024] op per m)
     outT[65,n] += v_aug[m].T @ P^T     (row 64 = rowsum via ones column)
     hT = outT[0:64] * bcast(1/rowsum)  (DVE mult; hT aliases the dead Q tile)
  phase 3: y = hT.T @ W_proj
"""
from contextlib import nullcontext

import numpy as np

import concourse.bass as bass
import concourse.mybir as mybir
import concourse.tile as tile
from concourse import bacc
from concourse.bass_utils import run_bass_kernel_spmd

F32R = mybir.dt.float32r
F32 = mybir.dt.float32

B, N, C = 8, 1024, 768
H, D = 12, 64
HID = H * D  # 768
KT = C // 128          # 6 feature k-tiles
MT = N // 128          # 8 sequence m-tiles
SCALE = D ** -0.5      # 0.125

_cached_nc = None

DEFAULT_OPTS = dict(
    s_bufs=2, acc_bufs=2, mm1_bufs=2, pt_bufs=4,
    eager_acc_evict=True, interleave_loads=True, proj_dual_pool=False,
    hoist_pair0=True,
)


def build_program(repeats=1, phases=("qk", "v", "attn", "proj"), **opts):
    o = dict(DEFAULT_OPTS, **opts)
    nc = bacc.Bacc(None, target_bir_lowering=False)

    xT_d = nc.dram_tensor("xT", [C, N], F32R, kind="ExternalInput")
    wqkv_d = nc.dram_tensor("wqkv", [C, 3 * HID], F32R, kind="ExternalInput")
    wproj_d = nc.dram_tensor("wproj", [HID, C], F32R, kind="ExternalInput")
    y_d = nc.dram_tensor("y", [N, C], F32, kind="ExternalOutput")

    with tile.TileContext(nc) as tc:
        with tc.tile_pool(name="persist", bufs=1) as persist, \
             tc.tile_pool(name="pt_pool", bufs=o["pt_bufs"]) as pt_pool, \
             tc.tile_pool(name="nrm_pool", bufs=3) as nrm_pool, \
             tc.tile_pool(name="y_pool", bufs=2) as y_pool, \
             tc.tile_pool(name="ps_a", bufs=o["mm1_bufs"], space="PSUM") as ps_a, \
             tc.tile_pool(name="ps_s", bufs=o["s_bufs"], space="PSUM") as ps_s, \
             tc.tile_pool(name="ps_acc", bufs=o["acc_bufs"], space="PSUM") as ps_acc:

            # ---- resident loads (emission order = DMA priority) ----
            xt = [persist.tile([128, N], F32R, name=f"xt{k}", tag=f"xt{k}")
                  for k in range(KT)]
            wqk = [persist.tile([128, 2 * HID], F32R, name=f"wqk{k}", tag=f"wqk{k}")
                   for k in range(KT)]
            wv = [persist.tile([128, HID], F32R, name=f"wv{k}", tag=f"wv{k}")
                  for k in range(KT)]
            if o["interleave_loads"]:
                for k in range(KT):
                    nc.sync.dma_start(xt[k][:], xT_d[k * 128:(k + 1) * 128, :])
                    nc.sync.dma_start(wqk[k][:],
                                      wqkv_d[k * 128:(k + 1) * 128, :2 * HID])
                for k in range(KT):
                    nc.sync.dma_start(wv[k][:], wqkv_d[k * 128:(k + 1) * 128, 2 * HID:])
            else:
                for k in range(KT):
                    nc.sync.dma_start(xt[k][:], xT_d[k * 128:(k + 1) * 128, :])
                for k in range(KT):
                    nc.sync.dma_start(wqk[k][:],
                                      wqkv_d[k * 128:(k + 1) * 128, :2 * HID])
                for k in range(KT):
                    nc.sync.dma_start(wv[k][:], wqkv_d[k * 128:(k + 1) * 128, 2 * HID:])

            # warm the exp table set during the DMA prefix (the ACT
            # table load otherwise lands on the first real exp)
            warm = persist.tile([1, 8], F32, name="warm", tag="warm")
            nc.gpsimd.memset(warm[:], 0.0)
            nc.scalar.activation(warm[:], warm[:],
                                 mybir.ActivationFunctionType.Exp)

            qkT = [persist.tile([128, N], F32R, name=f"qkT{t}", tag=f"qkT{t}")
                   for t in range(12)]
            v_aug = [persist.tile([128, H, D + 1], F32R, name=f"vaug{m}", tag=f"vaug{m}")
                     for m in range(MT)]
            hT = qkT[:6]  # normalized outputs overwrite the dead Q tiles

            # ---- phase 1a: one qkT tile (output rows = qkv cols t*128..) ----
            def qk_tile(t):
                for nh in range(2):
                    ps = ps_a.tile([128, 512], F32, name="ps_qk", tag="mm1")
                    for k in range(KT):
                        nc.tensor.matmul(ps[:], wqk[k][:, t * 128:(t + 1) * 128],
                                         xt[k][:, nh * 512:(nh + 1) * 512],
                                         start=(k == 0), stop=(k == KT - 1))
                    nc.vector.tensor_copy(qkT[t][:, nh * 512:(nh + 1) * 512], ps[:])

            # ---- phase 1b: v tiles ----
            def v_tile(m):
                for vh in range(2):
                    ps = ps_a.tile([128, 384], F32, name="ps_v", tag="mm1")
                    for k in range(KT):
                        nc.tensor.matmul(ps[:], xt[k][:, m * 128:(m + 1) * 128],
                                         wv[k][:, vh * 384:(vh + 1) * 384],
                                         start=(k == 0), stop=(k == KT - 1))
                    dst = v_aug[m][:, vh * 6:(vh + 1) * 6, 0:D]
                    nc.vector.tensor_copy(dst, ps[:].rearrange("p (h d) -> p h d", d=D))
                nc.gpsimd.memset(v_aug[m][:, :, D:D + 1].bitcast(F32), 1.0)

            # ---- phase 2: attention for head pair (2t, 2t+1) ----
            def attention(t, hoist=False):
                qT_t, kT_t = qkT[t], qkT[6 + t]
                for nh in range(2):
                    nsl = slice(nh * 512, (nh + 1) * 512)
                    acc = [ps_acc.tile([D + 1, 512], F32, name="acc", tag="acc")
                           for _ in range(2)]
                    for m in range(MT):
                        msl = slice(m * 128, (m + 1) * 128)
                        # both heads' S^T m-tile in one 2-bank psum tile;
                        # one [128,1024] exp serves both.
                        with tc.high_priority() if hoist else nullcontext():
                            s_ps = ps_s.tile([128, 1024], F32, name="s_ps", tag="s")
                            for j in range(2):
                                psl = slice(j * 64, (j + 1) * 64)
                                nc.tensor.matmul(s_ps[:, j * 512:(j + 1) * 512],
                                                 kT_t[psl, msl], qT_t[psl, nsl],
                                                 start=True, stop=True)
                            p_sb = pt_pool.tile([128, 1024], F32R, name="p_sb", tag="p")
                            nc.scalar.activation(p_sb[:], s_ps[:],
                                                 mybir.ActivationFunctionType.Exp,
                                                 scale=SCALE)
                        for j in range(2):
                            nc.tensor.matmul(acc[j][:], v_aug[m][:, 2 * t + j, :],
                                             p_sb[:, j * 512:(j + 1) * 512],
                                             start=(m == 0), stop=(m == MT - 1))
                    # normalize: rowsum sits in acc[j] row 64. HW
                    # partition_broadcast reads physical partition 0, so each
                    # reciprocal lives in its own [1, 512] tile.
                    for j in range(2):
                        rs = nrm_pool.tile([1, 512], F32, name="rs", tag="rs")
                        nc.vector.reciprocal(rs[0:1, :], acc[j][D:D + 1, :])
                        bc = nrm_pool.tile([64, 512], F32, name="bc", tag="bc")
                        nc.gpsimd.partition_broadcast(bc[:], rs[0:1, :])
                        if o["eager_acc_evict"]:
                            ev = pt_pool.tile([64, 512], F32, name="ev", tag="ev")
                            nc.vector.tensor_copy(ev[:], acc[j][0:D, :])
                            nc.vector.tensor_mul(hT[t][j * 64:(j + 1) * 64, nsl],
                                                 ev[:], bc[:])
                        else:
                            nc.vector.tensor_mul(hT[t][j * 64:(j + 1) * 64, nsl],
                                                 acc[j][0:D, :], bc[:])

            # ---- phase 3: y = hT.T @ W_proj ----
            def proj(m):
                for ph in range(2):
                    if o["proj_dual_pool"] and ph == 1:
                        ps = ps_s.tile([128, 384], F32, name="ps_y2", tag="s")
                    else:
                        ps = ps_a.tile([128, 384], F32, name="ps_y", tag="mm1")
                    for k in range(KT):
                        nc.tensor.matmul(ps[:], hT[k][:, m * 128:(m + 1) * 128],
                                         wp[k][:, ph * 384:(ph + 1) * 384],
                                         start=(k == 0), stop=(k == KT - 1))
                    y_sb = y_pool.tile([128, 384], F32, name="y_sb", tag="y")
                    if o.get("y_evict_dve"):
                        nc.vector.tensor_copy(y_sb[:], ps[:])
                    else:
                        nc.scalar.copy(y_sb[:], ps[:])
                    nc.sync.dma_start(
                        y_d[m * 128:(m + 1) * 128, ph * 384:(ph + 1) * 384], y_sb[:])

            for _ in range(repeats):
                if "qk" in phases:
                    qk_tile(0)
                    qk_tile(6)
                if "v" in phases:
                    for m in range(MT):
                        v_tile(m)
                if "qk" in phases and "attn" in phases:
                    # pair 0's S^T/exp get hoisted over the v-phase PE work
                    attention(0, hoist=o["hoist_pair0"])
                    for t in range(1, 6):
                        qk_tile(t)
                        qk_tile(6 + t)
                        attention(t)
                elif "qk" in phases:
                    for t in range(1, 6):
                        qk_tile(t)
                        qk_tile(6 + t)
                if "proj" in phases:
                    wp = [persist.tile([128, C], F32R, name=f"wp{k}", tag=f"wp{k}")
                          for k in range(KT)]
                    for k in range(KT):
                        nc.sync.dma_start(wp[k][:], wproj_d[k * 128:(k + 1) * 128, :])
                    for m in range(MT):
                        proj(m)

    nc.compile()
    return nc


def _run(inputs, trace=False, trace_kwargs=None):
    global _cached_nc
    x = np.asarray(inputs["x"], dtype=np.float32)
    wqkv = np.ascontiguousarray(np.asarray(inputs["W_qkv"], dtype=np.float32))
    wproj = np.ascontiguousarray(np.asarray(inputs["W_proj"], dtype=np.float32))
    xT = np.ascontiguousarray(x.transpose(0, 2, 1))  # [B, C, N]

    if _cached_nc is None:
        _cached_nc = build_program()
    nc = _cached_nc

    in_maps = [{"xT": xT[b], "wqkv": wqkv, "wproj": wproj} for b in range(B)]
    kwargs = {}
    if trace:
        kwargs["trace"] = True
        if trace_kwargs:
            kwargs.update(trace_kwargs)
    try:
        res = run_bass_kernel_spmd(nc, in_maps, core_ids=list(range(B)), **kwargs)
    except Exception:
        # transient axon/PJRT hiccups happen; one retry
        res = run_bass_kernel_spmd(nc, in_maps, core_ids=list(range(B)), **kwargs)
    out = np.stack([r["y"] for r in res.results], axis=0)
    return out, res


def kernel(**inputs):
    out, _ = _run(inputs)
    return out



# revision 22
# speedup vs baseline: 1.2138x; 1.2138x over previous
"""Multi-head attention (B=8, N=1024, H=12, D=64, C=768) on 8 trn2 cores.

Sharding: data-parallel over batch. Core b computes attention for x[b];
weights are replicated. No collectives.

Per-core dataflow:
  qkT[12][128,1024] bf16 : d-major Q^T/K^T   (f32r matmul, bf16 evict)
  v_bf[8][128,12,65] bf16: n-major V per m-tile + ones column (rowsum)
  per unit u=(nh,t) over 8 m-slots:
     S^T[m,n] = k^T.T @ q^T  (bf16, psum f32, 2 heads x 512 n)
     P^T = exp(S^T/8)        (ScalarE, bf16 out)
     prev unit's PV chunk:   acc[128 n, 65] += P^T-slice.T @ v_aug
                             (bf16 operands, 128 n-partitions: half the
                              PE cycles of the d-major form)
     prev norms:             recip(rowsum col) + per-partition scale (DVE)
  h[128 n, 128 dpair] --PE transpose--> hT[t][128, 1024] bf16
  y = hT.T @ W_proj (bf16) ; proj of n-half 0 overlaps nh=1 attention.
"""
import numpy as np

import concourse.bass as bass
import concourse.mybir as mybir
import concourse.tile as tile
from concourse import bacc
from concourse.bass_utils import run_bass_kernel_spmd
from concourse.masks import make_identity

F32R = mybir.dt.float32r
F32 = mybir.dt.float32
BF16 = mybir.dt.bfloat16
EXP = mybir.ActivationFunctionType.Exp

B, N, C = 8, 1024, 768
H, D = 12, 64
HID = H * D  # 768
KT = C // 128          # 6 feature k-tiles
MT = N // 128          # 8 sequence m-tiles
SCALE = D ** -0.5      # 0.125

_cached_nc = None


def build_program(debug=False):
    nc = bacc.Bacc(None, target_bir_lowering=False)

    xT_d = nc.dram_tensor("xT", [C, N], F32R, kind="ExternalInput")
    wqkv_d = nc.dram_tensor("wqkv", [C, 3 * HID], F32R, kind="ExternalInput")
    wproj_d = nc.dram_tensor("wproj", [HID, C], F32R, kind="ExternalInput")
    y_d = nc.dram_tensor("y", [N, C], F32, kind="ExternalOutput")
    if debug:
        dbg_qkT = nc.dram_tensor("dbg_qkT", [12, 128, N], BF16, kind="ExternalOutput")
        dbg_v = nc.dram_tensor("dbg_v", [MT, 128, H, D + 1], BF16,
                               kind="ExternalOutput")
        dbg_hT = nc.dram_tensor("dbg_hT", [KT, 128, N], BF16, kind="ExternalOutput")
        dbg_p = nc.dram_tensor("dbg_p", [128, 1024], BF16, kind="ExternalOutput")
        dbg_accA = nc.dram_tensor("dbg_accA", [128, 512], F32, kind="ExternalOutput")
        dbg_accB = nc.dram_tensor("dbg_accB", [128, 512], F32, kind="ExternalOutput")
        dbg_h = nc.dram_tensor("dbg_h", [4, 128, 128], BF16, kind="ExternalOutput")

    with tile.TileContext(nc) as tc:
        with tc.tile_pool(name="persist", bufs=1) as persist, \
             tc.tile_pool(name="pt_pool", bufs=16) as pt_pool, \
             tc.tile_pool(name="hsb_pool", bufs=10) as hsb_pool, \
             tc.tile_pool(name="nrm_pool", bufs=6) as nrm_pool, \
             tc.tile_pool(name="stage_pool", bufs=2) as stage_pool, \
             tc.tile_pool(name="y_pool", bufs=2) as y_pool, \
             tc.tile_pool(name="ps_a", bufs=2, space="PSUM") as ps_a, \
             tc.tile_pool(name="ps_s", bufs=2, space="PSUM") as ps_s, \
             tc.tile_pool(name="ps_acc", bufs=2, space="PSUM") as ps_acc:

            # ---- resident tiles ----
            xt = [persist.tile([128, N], F32R, name=f"xt{k}", tag=f"xt{k}")
                  for k in range(KT)]
            wqk = [persist.tile([128, 2 * HID], F32R, name=f"wqk{k}", tag=f"wqk{k}")
                   for k in range(KT)]
            wv = [persist.tile([128, HID], F32R, name=f"wv{k}", tag=f"wv{k}")
                  for k in range(KT)]
            wp = [persist.tile([128, C], BF16, name=f"wp{k}", tag=f"wp{k}")
                  for k in range(KT)]
            qkT = [persist.tile([128, N], BF16, name=f"qkT{t}", tag=f"qkT{t}")
                   for t in range(12)]
            v_bf = [persist.tile([128, H, D + 1], BF16, name=f"vbf{m}", tag=f"vbf{m}")
                    for m in range(MT)]
            hT = [persist.tile([128, N], BF16, name=f"hT{t}", tag=f"hT{t}")
                  for t in range(KT)]
            ident = persist.tile([128, 128], BF16, name="ident", tag="ident")

            # ---- DMA emission order = priority ----
            # Per k: x k-tile + the wqk column slices qk_tile(0)/(6) need, so
            # the first accumulation chain starts as soon as possible; then wv
            # (needed by v tiles right after S(u0)), then the rest.
            for k in range(KT):
                nc.sync.dma_start(xt[k][:], xT_d[k * 128:(k + 1) * 128, :])
                nc.sync.dma_start(wqk[k][:, 0:128],
                                  wqkv_d[k * 128:(k + 1) * 128, 0:128])
                nc.sync.dma_start(wqk[k][:, 768:896],
                                  wqkv_d[k * 128:(k + 1) * 128, 768:896])
            for k in range(KT):
                nc.sync.dma_start(wv[k][:], wqkv_d[k * 128:(k + 1) * 128, 2 * HID:])
            for k in range(KT):
                nc.sync.dma_start(wqk[k][:, 128:768],
                                  wqkv_d[k * 128:(k + 1) * 128, 128:768])
                nc.sync.dma_start(wqk[k][:, 896:1536],
                                  wqkv_d[k * 128:(k + 1) * 128, 896:1536])

            # warm the exp table during the DMA prefix
            warm = persist.tile([1, 8], F32, name="warm", tag="warm")
            nc.gpsimd.memset(warm[:], 0.0)
            nc.scalar.activation(warm[:], warm[:], EXP)
            make_identity(nc, ident[:])

            # ---- phase 1a: half a qkT tile (bf16 evict) ----
            def qk_half(t, nh):
                ps = ps_a.tile([128, 512], F32, name="ps_qk", tag="mm1")
                for k in range(KT):
                    nc.tensor.matmul(ps[:], wqk[k][:, t * 128:(t + 1) * 128],
                                     xt[k][:, nh * 512:(nh + 1) * 512],
                                     start=(k == 0), stop=(k == KT - 1))
                nc.vector.tensor_copy(qkT[t][:, nh * 512:(nh + 1) * 512], ps[:])

            # ---- phase 1b: v tiles (n-major, bf16, ones col) ----
            def v_tile(m):
                for vh in range(2):
                    ps = ps_a.tile([128, 384], F32, name="ps_v", tag="mm1")
                    for k in range(KT):
                        nc.tensor.matmul(ps[:], xt[k][:, m * 128:(m + 1) * 128],
                                         wv[k][:, vh * 384:(vh + 1) * 384],
                                         start=(k == 0), stop=(k == KT - 1))
                    dst = v_bf[m][:, vh * 6:(vh + 1) * 6, 0:D]
                    nc.vector.tensor_copy(dst, ps[:].rearrange("p (h d) -> p h d", d=D))
                nc.gpsimd.memset(v_bf[m][:, :, D:D + 1], 1.0)

            # ---- per-unit state ----
            ust = {}

            def pv_steps(u, half, ms):
                """PV accumulation steps `ms` for acc tile A (regions 0-3) or
                B (4-7) of unit u. Region c: gg=c//2 (n-subtile), j=c%2 (head
                in pair); regions live as 65-col strips at 128-col offsets.
                All of u's p tiles already exist when its PV runs (one unit
                later), so A can finish early — its norms then hide their DVE
                latency behind B's steps, and acc-tile ring reuse never
                stalls the next unit.
                """
                nh, t = u
                st = ust[u]
                key = "acc" + half
                if key not in st:
                    st[key] = ps_acc.tile([128, 512], F32, name="acc", tag="acc")
                acc = st[key]
                cs = range(4) if half == "A" else range(4, 8)
                for m in ms:
                    p = st["p"][m]
                    for c in cs:
                        gg, j = c // 2, c % 2
                        col = (c % 4) * 128
                        # start=True clears has_written for the whole psum
                        # bank row, so only the tile's first region may set
                        # it; the other regions' first step lands on cleared
                        # has_written and overwrites.
                        nc.tensor.matmul(acc[:, col:col + D + 1],
                                         p[:, j * 512 + gg * 128:
                                           j * 512 + (gg + 1) * 128],
                                         v_bf[m][:, 2 * t + j, :],
                                         start=(m == 0 and c % 4 == 0),
                                         stop=(m == MT - 1),
                                         skip_group_check=True)

            def norm_tile(u, half):
                """1/rowsum (col 64 of regions) * out -> h_sb[gg][:, j*64:]"""
                st = ust[u]
                acc = st["acc" + half]
                rs = nrm_pool.tile([128, 4], F32, name="rs", tag="rs")
                nc.vector.reciprocal(
                    rs[:], acc[:].rearrange("p (g c) -> p g c", c=128)[:, :, D])
                for ci in range(4):
                    c = ci if half == "A" else ci + 4
                    gg, j = c // 2, c % 2
                    col = ci * 128
                    if j == 0:
                        st["h"][gg] = hsb_pool.tile([128, 128], BF16,
                                                    name="h_sb", tag="h")
                    nc.vector.tensor_scalar_mul(st["h"][gg][:, j * 64:(j + 1) * 64],
                                                acc[:, col:col + D],
                                                rs[:, ci:ci + 1])

            pending_T = []   # (unit, gg) transposes deferred ~1 unit for slack

            def transpose_g(u, gg):
                nh, t = u
                st = ust[u]
                tp = ps_a.tile([128, 128], BF16, name="tp", tag="mm1")
                nc.tensor.transpose(tp[:], st["h"][gg][:], ident[:])
                g = nh * 4 + gg
                nc.vector.tensor_copy(hT[t][:, g * 128:(g + 1) * 128], tp[:])
                st["left"] -= 1
                if st["left"] == 0:
                    del ust[u]

            def s_phase(u, prev, fillers=None, prev_mmajor=False):
                """8 m-slots: S(u,m) + exp + prev's PV steps + filler work.

                prev's PV: acc A finishes by slot 3, its norms issue at slot 4
                (DVE latency hidden behind B's steps, so the acc ring never
                stalls the unit after); B finishes at slot 7, norms at end.
                prev_mmajor: step m at slot m for both tiles instead — u0's
                PV must wait for late v tiles that are still DMA-paced.
                Pending transposes (from the unit before prev) drop into even
                slots, a full unit after their norms — the Ldweights that
                loads h_sb never reaches PE.SEQ before its data is ready.
                Fillers keep per-slot PE work above the ScalarE exp pace so
                the 2-deep s_ps ring never throttles the pipeline.
                """
                nh, t = u
                fillers = fillers or {}
                ust[u] = {"p": [], "h": [None] * 4, "left": 4}
                for m in range(MT):
                    s_ps = ps_s.tile([128, 1024], F32, name="s_ps", tag="s")
                    for j in range(2):
                        psl = slice(j * 64, (j + 1) * 64)
                        nc.tensor.matmul(s_ps[:, j * 512:(j + 1) * 512],
                                         qkT[6 + t][psl, m * 128:(m + 1) * 128],
                                         qkT[t][psl, nh * 512:(nh + 1) * 512],
                                         start=True, stop=True)
                    p = pt_pool.tile([128, 1024], BF16, name="p_sb", tag="p")
                    nc.scalar.activation(p[:], s_ps[:], EXP, scale=SCALE)
                    if debug and u == (0, 0) and m == 0:
                        nc.sync.dma_start(dbg_p[:, :], p[:])
                    ust[u]["p"].append(p)
                    if prev is not None:
                        if prev_mmajor:
                            pv_steps(prev, "A", [m])
                            pv_steps(prev, "B", [m])
                        elif m < 4:
                            pv_steps(prev, "A", [2 * m, 2 * m + 1])
                        else:
                            pv_steps(prev, "B", [2 * (m - 4), 2 * (m - 4) + 1])
                        if m == 4 and not prev_mmajor:
                            norm_tile(prev, "A")
                    if m % 2 == 0 and pending_T:
                        transpose_g(*pending_T.pop(0))
                    for fn in fillers.get(m, ()):
                        fn()
                if prev is not None:
                    if debug and prev == (0, 0):
                        for nm, d_d in (("accA", dbg_accA), ("accB", dbg_accB)):
                            stg = stage_pool.tile([128, 512], F32, name="dbgs",
                                                  tag="wps")
                            nc.vector.tensor_copy(stg[:], ust[prev][nm][:])
                            nc.sync.dma_start(d_d[:, :], stg[:])
                    if prev_mmajor:
                        norm_tile(prev, "A")
                    norm_tile(prev, "B")
                    if debug and prev == (0, 0):
                        for gg in range(4):
                            nc.sync.dma_start(dbg_h[gg], ust[prev]["h"][gg][:])
                    for gg in range(4):
                        pending_T.append((prev, gg))

            # ---- phase 3: half a y tile ----
            def proj_half(m, ph, dve_evict=True):
                ps = ps_a.tile([128, 384], F32, name="ps_y", tag="mm1")
                for k in range(KT):
                    nc.tensor.matmul(ps[:], hT[k][:, m * 128:(m + 1) * 128],
                                     wp[k][:, ph * 384:(ph + 1) * 384],
                                     start=(k == 0), stop=(k == KT - 1))
                y_sb = y_pool.tile([128, 384], F32, name="y_sb", tag="y")
                if dve_evict:
                    nc.vector.tensor_copy(y_sb[:], ps[:])
                else:
                    nc.scalar.copy(y_sb[:], ps[:])   # tail: ScalarE is idle
                nc.sync.dma_start(
                    y_d[m * 128:(m + 1) * 128, ph * 384:(ph + 1) * 384], y_sb[:])

            def load_wp():
                for k in range(KT):
                    stg = stage_pool.tile([128, C], F32R, name="wps", tag="wps")
                    nc.sync.dma_start(stg[:], wproj_d[k * 128:(k + 1) * 128, :])
                    nc.vector.tensor_copy(wp[k][:], stg[:])

            # ---- emission schedule ----
            # Filler placement keeps every s_phase slot's PE work at or above
            # the ScalarE exp pace (EDF for qk halves: kT full + qT's working
            # half before a pair's first unit, the other qT half an nh-phase
            # later; v tiles late enough for their DMA; proj halves last).
            units = [(nh, t) for nh in range(2) for t in range(6)]
            qk_half(0, 0)
            qk_half(6, 0)
            qk_half(6, 1)
            s_phase(units[0], None, {4: [lambda: v_tile(0)],
                                     5: [lambda: v_tile(1)],
                                     6: [lambda: v_tile(2)],
                                     7: [lambda: v_tile(3)]})
            v_tile(4)
            qk_half(1, 0)
            qk_half(7, 0)
            qk_half(7, 1)
            F = {}
            F[1] = {0: [lambda: v_tile(5)], 1: [lambda: v_tile(6)],
                    2: [lambda: v_tile(7)], 3: [lambda: qk_half(2, 0)],
                    5: [lambda: qk_half(8, 0)], 6: [lambda: qk_half(8, 1)]}
            for i, tq in ((2, 3), (3, 4), (4, 5)):
                F[i] = {1: [lambda tq=tq: qk_half(tq, 0)],
                        3: [lambda tq=tq: qk_half(tq + 6, 0)],
                        5: [lambda tq=tq: qk_half(tq + 6, 1)]}
            F[5] = {1: [lambda: qk_half(0, 1)], 3: [load_wp]}
            F[6] = {1: [lambda: qk_half(1, 1)]}
            for i in (7, 8, 9):
                mp = i - 7
                F[i] = {1: [lambda i=i: qk_half(i - 5, 1)],
                        3: [lambda mp=mp: proj_half(mp, 0)],
                        5: [lambda mp=mp: proj_half(mp, 1)]}
            F[10] = {1: [lambda: qk_half(5, 1)], 3: [lambda: proj_half(3, 0)]}
            F[11] = {1: [lambda: proj_half(3, 1)]}
            for i in range(1, 12):
                s_phase(units[i], units[i - 1], F.get(i), prev_mmajor=(i == 1))
            # tail: last unit's PV / norms / transposes / proj, pipelined
            last = units[11]
            for m in range(4):
                pv_steps(last, "A", [2 * m, 2 * m + 1])
                if pending_T:                    # u10's deferred transposes
                    transpose_g(*pending_T.pop(0))
            norm_tile(last, "A")
            for m in range(4):
                pv_steps(last, "B", [2 * m, 2 * m + 1])
                if pending_T:
                    transpose_g(*pending_T.pop(0))
            norm_tile(last, "B")
            for gg in range(4):                  # u11's own transposes + proj
                transpose_g(last, gg)
                proj_half(4 + gg, 0, dve_evict=False)
                proj_half(4 + gg, 1, dve_evict=False)
            if debug:
                for t in range(12):
                    nc.sync.dma_start(dbg_qkT[t], qkT[t][:])
                for m in range(MT):
                    nc.sync.dma_start(dbg_v[m], v_bf[m][:])
                for k in range(KT):
                    nc.sync.dma_start(dbg_hT[k], hT[k][:])

    nc.compile()
    return nc


def _run(inputs, trace=False, trace_kwargs=None):
    global _cached_nc
    x = np.asarray(inputs["x"], dtype=np.float32)
    wqkv = np.ascontiguousarray(np.asarray(inputs["W_qkv"], dtype=np.float32))
    wproj = np.ascontiguousarray(np.asarray(inputs["W_proj"], dtype=np.float32))
    xT = np.ascontiguousarray(x.transpose(0, 2, 1))  # [B, C, N]

    if _cached_nc is None:
        _cached_nc = build_program()
    nc = _cached_nc

    in_maps = [{"xT": xT[b], "wqkv": wqkv, "wproj": wproj} for b in range(B)]
    kwargs = {}
    if trace:
        kwargs["trace"] = True
        if trace_kwargs:
            kwargs.update(trace_kwargs)
    try:
        res = run_bass_kernel_spmd(nc, in_maps, core_ids=list(range(B)), **kwargs)
    except Exception:
        # transient axon/PJRT hiccups happen; one retry
        res = run_bass_kernel_spmd(nc, in_maps, core_ids=list(range(B)), **kwargs)
    out = np.stack([r["y"] for r in res.results], axis=0)
    return out, res


def kernel(**inputs):
    out, _ = _run(inputs)
    return out


# revision 39
# speedup vs baseline: 1.2361x; 1.0184x over previous
"""Multi-head attention (B=8, N=1024, H=12, D=64, C=768) on 8 trn2 cores.

Sharding: data-parallel over batch. Core b computes attention for x[b];
weights are replicated. No collectives.

Per-core dataflow:
  qkT[12][128,1024] bf16 : d-major Q^T/K^T   (f32r matmul, bf16 evict)
  v_bf[8][128,12,65] bf16: n-major V per m-tile + ones column (rowsum)
  per unit u=(nh,t) over 8 m-slots:
     S^T[m,n] = k^T.T @ q^T  (bf16, psum f32, 2 heads x 512 n)
     P^T = exp(S^T/8)        (ScalarE, bf16 out)
     prev unit's PV chunk:   acc[128 n, 65] += P^T-slice.T @ v_aug
                             (bf16 operands, 128 n-partitions: half the
                              PE cycles of the d-major form)
     prev norms:             recip(rowsum col) + per-partition scale (DVE)
  h[128 n, 128 dpair] --PE transpose--> hT[t][128, 1024] bf16
  y = hT.T @ W_proj (bf16) ; proj of n-half 0 overlaps nh=1 attention.
"""
import numpy as np

import concourse.bass as bass
import concourse.mybir as mybir
import concourse.tile as tile
from concourse import bacc
from concourse.bass_utils import run_bass_kernel_spmd
from concourse.masks import make_identity

F32R = mybir.dt.float32r
F32 = mybir.dt.float32
BF16 = mybir.dt.bfloat16
EXP = mybir.ActivationFunctionType.Exp

B, N, C = 8, 1024, 768
H, D = 12, 64
HID = H * D  # 768
KT = C // 128          # 6 feature k-tiles
MT = N // 128          # 8 sequence m-tiles
SCALE = D ** -0.5      # 0.125

_cached_nc = None


def build_program(debug=False):
    nc = bacc.Bacc(None, target_bir_lowering=False)

    xT_d = nc.dram_tensor("xT", [C, N], F32R, kind="ExternalInput")
    wqkv_d = nc.dram_tensor("wqkv", [C, 3 * HID], F32R, kind="ExternalInput")
    wproj_d = nc.dram_tensor("wproj", [HID, C], F32R, kind="ExternalInput")
    y_d = nc.dram_tensor("y", [N, C], F32, kind="ExternalOutput")
    if debug:
        dbg_qkT = nc.dram_tensor("dbg_qkT", [12, 128, N], BF16, kind="ExternalOutput")
        dbg_v = nc.dram_tensor("dbg_v", [MT, 128, H, D + 1], BF16,
                               kind="ExternalOutput")
        dbg_hT = nc.dram_tensor("dbg_hT", [KT, 128, N], BF16, kind="ExternalOutput")
        dbg_p = nc.dram_tensor("dbg_p", [128, 1024], BF16, kind="ExternalOutput")
        dbg_accA = nc.dram_tensor("dbg_accA", [128, 512], F32, kind="ExternalOutput")
        dbg_accB = nc.dram_tensor("dbg_accB", [128, 512], F32, kind="ExternalOutput")
        dbg_h = nc.dram_tensor("dbg_h", [4, 128, 128], BF16, kind="ExternalOutput")

    with tile.TileContext(nc) as tc:
        with tc.tile_pool(name="persist", bufs=1) as persist, \
             tc.tile_pool(name="pt_pool", bufs=16) as pt_pool, \
             tc.tile_pool(name="hsb_pool", bufs=10) as hsb_pool, \
             tc.tile_pool(name="nrm_pool", bufs=6) as nrm_pool, \
             tc.tile_pool(name="stage_pool", bufs=2) as stage_pool, \
             tc.tile_pool(name="y_pool", bufs=2) as y_pool, \
             tc.tile_pool(name="ps_a", bufs=2, space="PSUM") as ps_a, \
             tc.tile_pool(name="ps_s", bufs=2, space="PSUM") as ps_s, \
             tc.tile_pool(name="ps_acc", bufs=2, space="PSUM") as ps_acc:

            # ---- resident tiles ----
            xt = [persist.tile([128, N], F32R, name=f"xt{k}", tag=f"xt{k}")
                  for k in range(KT)]
            wqk = [persist.tile([128, 2 * HID], F32R, name=f"wqk{k}", tag=f"wqk{k}")
                   for k in range(KT)]
            wv = [persist.tile([128, HID], F32R, name=f"wv{k}", tag=f"wv{k}")
                  for k in range(KT)]
            wp = [persist.tile([128, C], BF16, name=f"wp{k}", tag=f"wp{k}")
                  for k in range(KT)]
            qkT = [persist.tile([128, N], BF16, name=f"qkT{t}", tag=f"qkT{t}")
                   for t in range(12)]
            v_bf = [persist.tile([128, H, D + 1], BF16, name=f"vbf{m}", tag=f"vbf{m}")
                    for m in range(MT)]
            hT = [persist.tile([128, N], BF16, name=f"hT{t}", tag=f"hT{t}")
                  for t in range(KT)]
            ident = persist.tile([128, 128], BF16, name="ident", tag="ident")

            # ---- DMA emission order = priority ----
            # qk_half(0,0)/(6,*) need the t0/t6 wqk column slices + x; x's
            # n-halves split so the first S chain starts off xt-h0 alone.
            # wv lands in time for v tiles in u0's late slots; t1/t7 slices
            # before the bulk so unit (0,1) can start; rest streams after.
            for k in range(KT):
                nc.sync.dma_start(wqk[k][:, 0:128],     # t0 slice first: tiny,
                                  wqkv_d[k * 128:(k + 1) * 128, 0:128])
                nc.sync.dma_start(xt[k][:, 0:512],      # so SP's serial issue
                                  xT_d[k * 128:(k + 1) * 128, 0:512])
                nc.sync.dma_start(wqk[k][:, 768:896],   # doesn't gate mm #1
                                  wqkv_d[k * 128:(k + 1) * 128, 768:896])
            for k in range(KT):
                nc.sync.dma_start(xt[k][:, 512:1024],
                                  xT_d[k * 128:(k + 1) * 128, 512:1024])
            for k in range(KT):
                nc.sync.dma_start(wv[k][:], wqkv_d[k * 128:(k + 1) * 128, 2 * HID:])
            for k in range(KT):
                for c0 in (128, 896):            # t1, t7 col slices
                    nc.sync.dma_start(wqk[k][:, c0:c0 + 128],
                                      wqkv_d[k * 128:(k + 1) * 128, c0:c0 + 128])
            for k in range(KT):
                nc.sync.dma_start(wqk[k][:, 256:768],
                                  wqkv_d[k * 128:(k + 1) * 128, 256:768])
                nc.sync.dma_start(wqk[k][:, 1024:1536],
                                  wqkv_d[k * 128:(k + 1) * 128, 1024:1536])

            # warm the exp table during the DMA prefix
            warm = persist.tile([1, 8], F32, name="warm", tag="warm")
            nc.gpsimd.memset(warm[:], 0.0)
            nc.scalar.activation(warm[:], warm[:], EXP)
            make_identity(nc, ident[:])

            # ---- phase 1a: half a qkT tile (bf16 evict) ----
            def qk_half(t, nh):
                ps = ps_a.tile([128, 512], F32, name="ps_qk", tag="mm1")
                for k in range(KT):
                    nc.tensor.matmul(ps[:], wqk[k][:, t * 128:(t + 1) * 128],
                                     xt[k][:, nh * 512:(nh + 1) * 512],
                                     start=(k == 0), stop=(k == KT - 1))
                nc.vector.tensor_copy(qkT[t][:, nh * 512:(nh + 1) * 512], ps[:])

            def qk_headgroup(specs):
                """k-step-major interleave of several qk chains, so each
                arriving xt k-tile immediately feeds every chain (the head
                is DMA-paced; chain-major would idle PE between k-tiles).
                ps_s is idle this early — borrow it for the extra chains.
                """
                states = []
                for ci, (t, nh) in enumerate(specs):
                    if ci < 2:
                        ps = ps_a.tile([128, 512], F32, name="ps_qk", tag="mm1")
                    else:
                        ps = ps_s.tile([128, 1024], F32, name="s_ps",
                                       tag="s")[:, 0:512]
                    states.append((t, nh, ps))
                for k in range(KT):
                    for t, nh, ps in states:
                        nc.tensor.matmul(ps, wqk[k][:, t * 128:(t + 1) * 128],
                                         xt[k][:, nh * 512:(nh + 1) * 512],
                                         start=(k == 0), stop=(k == KT - 1))
                for t, nh, ps in states:
                    nc.vector.tensor_copy(qkT[t][:, nh * 512:(nh + 1) * 512], ps)

            # ---- phase 1b: v half-tiles (n-major, bf16, ones col) ----
            def v_half(m, vh):
                ps = ps_a.tile([128, 384], F32, name="ps_v", tag="mm1")
                for k in range(KT):
                    nc.tensor.matmul(ps[:], xt[k][:, m * 128:(m + 1) * 128],
                                     wv[k][:, vh * 384:(vh + 1) * 384],
                                     start=(k == 0), stop=(k == KT - 1))
                dst = v_bf[m][:, vh * 6:(vh + 1) * 6, 0:D]
                nc.vector.tensor_copy(dst, ps[:].rearrange("p (h d) -> p h d", d=D))
                if vh == 1:
                    nc.gpsimd.memset(v_bf[m][:, :, D:D + 1], 1.0)

            # ---- per-unit state ----
            ust = {}

            def pv_steps(u, half, ms):
                """PV accumulation steps `ms` for acc tile A (regions 0-3) or
                B (4-7) of unit u. Region c: gg=c//2 (n-subtile), j=c%2 (head
                in pair); regions live as 65-col strips at 128-col offsets.
                All of u's p tiles already exist when its PV runs (one unit
                later), so A can finish early — its norms then hide their DVE
                latency behind B's steps, and acc-tile ring reuse never
                stalls the next unit.
                """
                nh, t = u
                st = ust[u]
                key = "acc" + half
                if key not in st:
                    st[key] = ps_acc.tile([128, 512], F32, name="acc", tag="acc")
                acc = st[key]
                cs = range(4) if half == "A" else range(4, 8)
                for m in ms:
                    p = st["p"][m]
                    for c in cs:
                        gg, j = c // 2, c % 2
                        col = (c % 4) * 128
                        # start=True clears has_written for the whole psum
                        # bank row, so only the tile's first region may set
                        # it; the other regions' first step lands on cleared
                        # has_written and overwrites.
                        nc.tensor.matmul(acc[:, col:col + D + 1],
                                         p[:, j * 512 + gg * 128:
                                           j * 512 + (gg + 1) * 128],
                                         v_bf[m][:, 2 * t + j, :],
                                         start=(m == 0 and c % 4 == 0),
                                         stop=(m == MT - 1),
                                         skip_group_check=True)

            def norm_tile(u, half):
                """1/rowsum (col 64 of regions) * out -> h_sb[gg][:, j*64:]"""
                st = ust[u]
                acc = st["acc" + half]
                rs = nrm_pool.tile([128, 4], F32, name="rs", tag="rs")
                nc.vector.reciprocal(
                    rs[:], acc[:].rearrange("p (g c) -> p g c", c=128)[:, :, D])
                for ci in range(4):
                    c = ci if half == "A" else ci + 4
                    gg, j = c // 2, c % 2
                    col = ci * 128
                    if j == 0:
                        st["h"][gg] = hsb_pool.tile([128, 128], BF16,
                                                    name="h_sb", tag="h")
                    nc.vector.tensor_scalar_mul(st["h"][gg][:, j * 64:(j + 1) * 64],
                                                acc[:, col:col + D],
                                                rs[:, ci:ci + 1])

            pending_T = []   # (unit, gg) transposes deferred ~1 unit for slack

            def transpose_g(u, gg):
                nh, t = u
                st = ust[u]
                tp = ps_a.tile([128, 128], BF16, name="tp", tag="mm1")
                nc.tensor.transpose(tp[:], st["h"][gg][:], ident[:])
                g = nh * 4 + gg
                nc.vector.tensor_copy(hT[t][:, g * 128:(g + 1) * 128], tp[:])
                st["left"] -= 1
                if st["left"] == 0:
                    del ust[u]

            def s_phase(u, prev, fillers=None, prev_mmajor=False):
                """8 m-slots: S(u,m) + exp + prev's PV steps + filler work.

                prev's PV: acc A finishes by slot 3, its norms issue at slot 4
                (DVE latency hidden behind B's steps, so the acc ring never
                stalls the unit after); B finishes at slot 7, norms at end.
                prev_mmajor: step m at slot m for both tiles instead — u0's
                PV must wait for late v tiles that are still DMA-paced.
                Pending transposes (from the unit before prev) drop into even
                slots, a full unit after their norms — the Ldweights that
                loads h_sb never reaches PE.SEQ before its data is ready.
                Fillers keep per-slot PE work above the ScalarE exp pace so
                the 2-deep s_ps ring never throttles the pipeline.
                """
                nh, t = u
                fillers = fillers or {}
                ust[u] = {"p": [], "h": [None] * 4, "left": 4}
                for m in range(MT):
                    s_ps = ps_s.tile([128, 1024], F32, name="s_ps", tag="s")
                    for j in range(2):
                        psl = slice(j * 64, (j + 1) * 64)
                        nc.tensor.matmul(s_ps[:, j * 512:(j + 1) * 512],
                                         qkT[6 + t][psl, m * 128:(m + 1) * 128],
                                         qkT[t][psl, nh * 512:(nh + 1) * 512],
                                         start=True, stop=True)
                    p = pt_pool.tile([128, 1024], BF16, name="p_sb", tag="p")
                    nc.scalar.activation(p[:], s_ps[:], EXP, scale=SCALE)
                    if debug and u == (0, 0) and m == 0:
                        nc.sync.dma_start(dbg_p[:, :], p[:])
                    ust[u]["p"].append(p)
                    if prev is not None:
                        if prev_mmajor:
                            pv_steps(prev, "A", [m])
                            pv_steps(prev, "B", [m])
                        elif m < 4:
                            pv_steps(prev, "A", [2 * m, 2 * m + 1])
                        else:
                            pv_steps(prev, "B", [2 * (m - 4), 2 * (m - 4) + 1])
                        if m == 4 and not prev_mmajor:
                            norm_tile(prev, "A")
                    if m % 2 == 0 and pending_T:
                        transpose_g(*pending_T.pop(0))
                    for fn in fillers.get(m, ()):
                        fn()
                if prev is not None:
                    if debug and prev == (0, 0):
                        for nm, d_d in (("accA", dbg_accA), ("accB", dbg_accB)):
                            stg = stage_pool.tile([128, 512], F32, name="dbgs",
                                                  tag="wps")
                            nc.vector.tensor_copy(stg[:], ust[prev][nm][:])
                            nc.sync.dma_start(d_d[:, :], stg[:])
                    if prev_mmajor:
                        norm_tile(prev, "A")
                    norm_tile(prev, "B")
                    if debug and prev == (0, 0):
                        for gg in range(4):
                            nc.sync.dma_start(dbg_h[gg], ust[prev]["h"][gg][:])
                    for gg in range(4):
                        pending_T.append((prev, gg))

            # ---- phase 3: half a y tile; one contiguous DMA per full tile
            # (half-tile stores cost ~1-2.5us each on SP.SEQ descriptor gen)
            y_tiles = {}

            def proj_half(m, ph, dve_evict=True):
                ps = ps_a.tile([128, 384], F32, name="ps_y", tag="mm1")
                for k in range(KT):
                    nc.tensor.matmul(ps[:], hT[k][:, m * 128:(m + 1) * 128],
                                     wp[k][:, ph * 384:(ph + 1) * 384],
                                     start=(k == 0), stop=(k == KT - 1))
                if ph == 0:
                    y_tiles[m] = y_pool.tile([128, C], F32, name="y_sb", tag="y")
                y_sb = y_tiles[m]
                if dve_evict:
                    nc.vector.tensor_copy(y_sb[:, ph * 384:(ph + 1) * 384], ps[:])
                else:
                    nc.scalar.copy(y_sb[:, ph * 384:(ph + 1) * 384], ps[:])
                if ph == 1:
                    nc.sync.dma_start(y_d[m * 128:(m + 1) * 128, :], y_sb[:])

            def load_wp():
                for k in range(KT):
                    stg = stage_pool.tile([128, C], F32R, name="wps", tag="wps")
                    nc.sync.dma_start(stg[:], wproj_d[k * 128:(k + 1) * 128, :])
                    nc.vector.tensor_copy(wp[k][:], stg[:])

            # ---- emission schedule ----
            # Unit order: (0,0), (1,0) — the second costs no new weight DMA
            # (q0h1 computes from the already-loaded t0 slices), keeping the
            # exp stream alive while v tiles cook — then nh0's remaining t
            # (so n-half-0 proj can overlap late nh1 units), then nh1's.
            # Filler placement keeps every s_phase slot's PE work at or above
            # the ScalarE exp pace (EDF for qk halves: kT full + qT's working
            # half before a pair's first unit, the other qT half before the
            # pair's nh1 unit; v tiles late enough for their DMA; proj last).
            units = ([(0, 0), (1, 0)] + [(0, t) for t in range(1, 6)]
                     + [(1, t) for t in range(1, 6)])
            qk_headgroup([(0, 0), (6, 0)])                   # xt-h0 paced
            qk_headgroup([(6, 1), (0, 1)])                   # xt-h1 paced
            s_phase(units[0], None, {
                4: [lambda: v_half(0, 0), lambda: v_half(0, 1)],
                5: [lambda: v_half(1, 0), lambda: v_half(1, 1)],
                6: [lambda: v_half(2, 0), lambda: v_half(2, 1)],
                7: [lambda: v_half(3, 0), lambda: v_half(3, 1)]})
            F = {}
            F[1] = {0: [lambda: v_half(4, 0), lambda: v_half(4, 1)],
                    1: [lambda: v_half(5, 0), lambda: v_half(5, 1)],
                    2: [lambda: v_half(6, 0), lambda: v_half(6, 1)],
                    3: [lambda: v_half(7, 0), lambda: v_half(7, 1)],
                    4: [lambda: qk_half(1, 0)], 5: [lambda: qk_half(7, 0)],
                    6: [lambda: qk_half(7, 1)]}
            for i, tq in ((2, 2), (3, 3), (4, 4), (5, 5)):
                F[i] = {1: [lambda tq=tq: qk_half(tq, 0)],
                        3: [lambda tq=tq: qk_half(tq + 6, 0)],
                        5: [lambda tq=tq: qk_half(tq + 6, 1)]}
            F[6] = {1: [lambda: qk_half(1, 1)], 3: [load_wp]}
            F[7] = {1: [lambda: qk_half(2, 1)]}
            for i in (8, 9, 10):
                mp = i - 8
                F[i] = {1: [lambda i=i: qk_half(i - 5, 1)],
                        3: [lambda mp=mp: proj_half(mp, 0)],
                        5: [lambda mp=mp: proj_half(mp, 1)]}
            F[11] = {1: [lambda: proj_half(3, 0)], 3: [lambda: proj_half(3, 1)]}
            for i in range(1, 12):
                s_phase(units[i], units[i - 1], F.get(i), prev_mmajor=(i == 1))
            # tail: last unit's PV / norms / transposes / proj, pipelined
            # (transposes run one g ahead of proj so the hT evict's DVE
            # latency hides under the previous proj's matmuls)
            last = units[11]
            for m in range(4):
                pv_steps(last, "A", [2 * m, 2 * m + 1])
                if pending_T:                    # u10's deferred transposes
                    transpose_g(*pending_T.pop(0))
            norm_tile(last, "A")
            for m in range(4):
                pv_steps(last, "B", [2 * m, 2 * m + 1])
                if pending_T:
                    transpose_g(*pending_T.pop(0))
            norm_tile(last, "B")
            transpose_g(last, 0)
            transpose_g(last, 1)
            for gg in range(4):
                if gg < 2:
                    transpose_g(last, gg + 2)
                proj_half(4 + gg, 0, dve_evict=False)
                proj_half(4 + gg, 1, dve_evict=False)
            if debug:
                for t in range(12):
                    nc.sync.dma_start(dbg_qkT[t], qkT[t][:])
                for m in range(MT):
                    nc.sync.dma_start(dbg_v[m], v_bf[m][:])
                for k in range(KT):
                    nc.sync.dma_start(dbg_hT[k], hT[k][:])

    nc.compile()
    return nc


def _run(inputs, trace=False, trace_kwargs=None):
    global _cached_nc
    x = np.asarray(inputs["x"], dtype=np.float32)
    wqkv = np.ascontiguousarray(np.asarray(inputs["W_qkv"], dtype=np.float32))
    wproj = np.ascontiguousarray(np.asarray(inputs["W_proj"], dtype=np.float32))
    xT = np.ascontiguousarray(x.transpose(0, 2, 1))  # [B, C, N]

    if _cached_nc is None:
        _cached_nc = build_program()
    nc = _cached_nc

    in_maps = [{"xT": xT[b], "wqkv": wqkv, "wproj": wproj} for b in range(B)]
    kwargs = {}
    if trace:
        kwargs["trace"] = True
        if trace_kwargs:
            kwargs.update(trace_kwargs)
    try:
        res = run_bass_kernel_spmd(nc, in_maps, core_ids=list(range(B)), **kwargs)
    except Exception:
        # transient axon/PJRT hiccups happen; one retry
        res = run_bass_kernel_spmd(nc, in_maps, core_ids=list(range(B)), **kwargs)
    out = np.stack([r["y"] for r in res.results], axis=0)
    return out, res


def kernel(**inputs):
    out, _ = _run(inputs)
    return out


# revision 43
# speedup vs baseline: 1.2420x; 1.0048x over previous
"""Multi-head attention (B=8, N=1024, H=12, D=64, C=768) on 8 trn2 cores.

Sharding: data-parallel over batch. Core b computes attention for x[b];
weights are replicated. No collectives.

Per-core dataflow:
  qkT[12][128,1024] bf16 : d-major Q^T/K^T   (f32r matmul, bf16 evict)
  v_bf[8][128,12,65] bf16: n-major V per m-tile + ones column (rowsum)
  per unit u=(nh,t) over 8 m-slots:
     S^T[m,n] = k^T.T @ q^T  (bf16, psum f32, 2 heads x 512 n)
     P^T = exp(S^T/8)        (ScalarE, bf16 out)
     prev unit's PV chunk:   acc[128 n, 65] += P^T-slice.T @ v_aug
                             (bf16 operands, 128 n-partitions: half the
                              PE cycles of the d-major form)
     prev norms:             recip(rowsum col) + per-partition scale (DVE)
  h[128 n, 128 dpair] --PE transpose--> hT[t][128, 1024] bf16
  y = hT.T @ W_proj (bf16) ; proj of n-half 0 overlaps nh=1 attention.
"""
import numpy as np

import concourse.bass as bass
import concourse.mybir as mybir
import concourse.tile as tile
from concourse import bacc
from concourse.bass_utils import run_bass_kernel_spmd
from concourse.masks import make_identity

F32R = mybir.dt.float32r
F32 = mybir.dt.float32
BF16 = mybir.dt.bfloat16
EXP = mybir.ActivationFunctionType.Exp

B, N, C = 8, 1024, 768
H, D = 12, 64
HID = H * D  # 768
KT = C // 128          # 6 feature k-tiles
MT = N // 128          # 8 sequence m-tiles
SCALE = D ** -0.5      # 0.125

_cached_nc = None


def build_program(debug=False):
    nc = bacc.Bacc(None, target_bir_lowering=False)

    xT_d = nc.dram_tensor("xT", [C, N], F32R, kind="ExternalInput")
    wqkv_d = nc.dram_tensor("wqkv", [C, 3 * HID], F32R, kind="ExternalInput")
    wproj_d = nc.dram_tensor("wproj", [HID, C], F32R, kind="ExternalInput")
    y_d = nc.dram_tensor("y", [N, C], F32, kind="ExternalOutput")
    if debug:
        dbg_qkT = nc.dram_tensor("dbg_qkT", [12, 128, N], BF16, kind="ExternalOutput")
        dbg_v = nc.dram_tensor("dbg_v", [MT, 128, H, D + 1], BF16,
                               kind="ExternalOutput")
        dbg_hT = nc.dram_tensor("dbg_hT", [KT, 128, N], BF16, kind="ExternalOutput")
        dbg_p = nc.dram_tensor("dbg_p", [128, 1024], BF16, kind="ExternalOutput")
        dbg_accA = nc.dram_tensor("dbg_accA", [128, 512], F32, kind="ExternalOutput")
        dbg_accB = nc.dram_tensor("dbg_accB", [128, 512], F32, kind="ExternalOutput")
        dbg_h = nc.dram_tensor("dbg_h", [4, 128, 128], BF16, kind="ExternalOutput")

    with tile.TileContext(nc) as tc:
        with tc.tile_pool(name="persist", bufs=1) as persist, \
             tc.tile_pool(name="pt_pool", bufs=18) as pt_pool, \
             tc.tile_pool(name="hsb_pool", bufs=12) as hsb_pool, \
             tc.tile_pool(name="nrm_pool", bufs=8) as nrm_pool, \
             tc.tile_pool(name="stage_pool", bufs=2) as stage_pool, \
             tc.tile_pool(name="y_pool", bufs=3) as y_pool, \
             tc.tile_pool(name="ps_a", bufs=2, space="PSUM") as ps_a, \
             tc.tile_pool(name="ps_s", bufs=2, space="PSUM") as ps_s, \
             tc.tile_pool(name="ps_acc", bufs=2, space="PSUM") as ps_acc:

            # ---- resident tiles ----
            xt = [persist.tile([128, N], F32R, name=f"xt{k}", tag=f"xt{k}")
                  for k in range(KT)]
            wqk = [persist.tile([128, 2 * HID], F32R, name=f"wqk{k}", tag=f"wqk{k}")
                   for k in range(KT)]
            wv = [persist.tile([128, HID], F32R, name=f"wv{k}", tag=f"wv{k}")
                  for k in range(KT)]
            wp = [persist.tile([128, C], BF16, name=f"wp{k}", tag=f"wp{k}")
                  for k in range(KT)]
            qkT = [persist.tile([128, N], BF16, name=f"qkT{t}", tag=f"qkT{t}")
                   for t in range(12)]
            v_bf = [persist.tile([128, H, D + 1], BF16, name=f"vbf{m}", tag=f"vbf{m}")
                    for m in range(MT)]
            hT = [persist.tile([128, N], BF16, name=f"hT{t}", tag=f"hT{t}")
                  for t in range(KT)]
            ident = persist.tile([128, 128], BF16, name="ident", tag="ident")

            # ---- DMA emission order = priority ----
            # qk_half(0,0)/(6,*) need the t0/t6 wqk column slices + x; x's
            # n-halves split so the first S chain starts off xt-h0 alone.
            # wv lands in time for v tiles in u0's late slots; t1/t7 slices
            # before the bulk so unit (0,1) can start; rest streams after.
            for k in range(KT):
                nc.sync.dma_start(xt[k][:, 0:512],
                                  xT_d[k * 128:(k + 1) * 128, 0:512])
                for c0 in (0, 768):              # t0, t6 col slices
                    nc.sync.dma_start(wqk[k][:, c0:c0 + 128],
                                      wqkv_d[k * 128:(k + 1) * 128, c0:c0 + 128])
            for k in range(KT):
                nc.sync.dma_start(xt[k][:, 512:1024],
                                  xT_d[k * 128:(k + 1) * 128, 512:1024])
            for k in range(KT):
                nc.sync.dma_start(wv[k][:], wqkv_d[k * 128:(k + 1) * 128, 2 * HID:])
            for k in range(KT):
                for c0 in (128, 896):            # t1, t7 col slices
                    nc.sync.dma_start(wqk[k][:, c0:c0 + 128],
                                      wqkv_d[k * 128:(k + 1) * 128, c0:c0 + 128])
            for k in range(KT):
                nc.sync.dma_start(wqk[k][:, 256:768],
                                  wqkv_d[k * 128:(k + 1) * 128, 256:768])
                nc.sync.dma_start(wqk[k][:, 1024:1536],
                                  wqkv_d[k * 128:(k + 1) * 128, 1024:1536])

            # warm the exp table during the DMA prefix
            warm = persist.tile([1, 8], F32, name="warm", tag="warm")
            nc.gpsimd.memset(warm[:], 0.0)
            nc.scalar.activation(warm[:], warm[:], EXP)
            make_identity(nc, ident[:])

            # ---- phase 1a: half a qkT tile (bf16 evict) ----
            def qk_half(t, nh):
                ps = ps_a.tile([128, 512], F32, name="ps_qk", tag="mm1")
                for k in range(KT):
                    nc.tensor.matmul(ps[:], wqk[k][:, t * 128:(t + 1) * 128],
                                     xt[k][:, nh * 512:(nh + 1) * 512],
                                     start=(k == 0), stop=(k == KT - 1))
                nc.vector.tensor_copy(qkT[t][:, nh * 512:(nh + 1) * 512], ps[:])

            def qk_headgroup(specs):
                """k-step-major interleave of several qk chains, so each
                arriving xt k-tile immediately feeds every chain (the head
                is DMA-paced; chain-major would idle PE between k-tiles).
                ps_s is idle this early — borrow it for the extra chains.
                """
                states = []
                for ci, (t, nh) in enumerate(specs):
                    if ci < 2:
                        ps = ps_a.tile([128, 512], F32, name="ps_qk", tag="mm1")
                    else:
                        ps = ps_s.tile([128, 1024], F32, name="s_ps",
                                       tag="s")[:, 0:512]
                    states.append((t, nh, ps))
                for k in range(KT):
                    for t, nh, ps in states:
                        nc.tensor.matmul(ps, wqk[k][:, t * 128:(t + 1) * 128],
                                         xt[k][:, nh * 512:(nh + 1) * 512],
                                         start=(k == 0), stop=(k == KT - 1))
                for t, nh, ps in states:
                    nc.vector.tensor_copy(qkT[t][:, nh * 512:(nh + 1) * 512], ps)

            # ---- phase 1b: v half-tiles (n-major, bf16, ones col) ----
            def v_half(m, vh):
                ps = ps_a.tile([128, 384], F32, name="ps_v", tag="mm1")
                for k in range(KT):
                    nc.tensor.matmul(ps[:], xt[k][:, m * 128:(m + 1) * 128],
                                     wv[k][:, vh * 384:(vh + 1) * 384],
                                     start=(k == 0), stop=(k == KT - 1))
                dst = v_bf[m][:, vh * 6:(vh + 1) * 6, 0:D]
                nc.vector.tensor_copy(dst, ps[:].rearrange("p (h d) -> p h d", d=D))
                if vh == 1:
                    nc.gpsimd.memset(v_bf[m][:, :, D:D + 1], 1.0)

            # ---- per-unit state ----
            ust = {}

            def pv_steps(u, half, ms):
                """PV accumulation steps `ms` for acc tile A (regions 0-3) or
                B (4-7) of unit u. Region c: gg=c//2 (n-subtile), j=c%2 (head
                in pair); regions live as 65-col strips at 128-col offsets.
                All of u's p tiles already exist when its PV runs (one unit
                later), so A can finish early — its norms then hide their DVE
                latency behind B's steps, and acc-tile ring reuse never
                stalls the next unit.
                """
                nh, t = u
                st = ust[u]
                key = "acc" + half
                if key not in st:
                    st[key] = ps_acc.tile([128, 512], F32, name="acc", tag="acc")
                acc = st[key]
                cs = range(4) if half == "A" else range(4, 8)
                for m in ms:
                    p = st["p"][m]
                    for c in cs:
                        gg, j = c // 2, c % 2
                        col = (c % 4) * 128
                        # start=True clears has_written for the whole psum
                        # bank row, so only the tile's first region may set
                        # it; the other regions' first step lands on cleared
                        # has_written and overwrites.
                        nc.tensor.matmul(acc[:, col:col + D + 1],
                                         p[:, j * 512 + gg * 128:
                                           j * 512 + (gg + 1) * 128],
                                         v_bf[m][:, 2 * t + j, :],
                                         start=(m == 0 and c % 4 == 0),
                                         stop=(m == MT - 1),
                                         skip_group_check=True)

            def norm_tile(u, half):
                """1/rowsum (col 64 of regions) * out -> h_sb[gg][:, j*64:]"""
                st = ust[u]
                acc = st["acc" + half]
                rs = nrm_pool.tile([128, 4], F32, name="rs", tag="rs")
                nc.vector.reciprocal(
                    rs[:], acc[:].rearrange("p (g c) -> p g c", c=128)[:, :, D])
                for ci in range(4):
                    c = ci if half == "A" else ci + 4
                    gg, j = c // 2, c % 2
                    col = ci * 128
                    if j == 0:
                        st["h"][gg] = hsb_pool.tile([128, 128], BF16,
                                                    name="h_sb", tag="h")
                    nc.vector.tensor_scalar_mul(st["h"][gg][:, j * 64:(j + 1) * 64],
                                                acc[:, col:col + D],
                                                rs[:, ci:ci + 1])

            pending_T = []   # (unit, gg) transposes deferred ~1 unit for slack

            def transpose_g(u, gg):
                nh, t = u
                st = ust[u]
                tp = ps_a.tile([128, 128], BF16, name="tp", tag="mm1")
                nc.tensor.transpose(tp[:], st["h"][gg][:], ident[:])
                g = nh * 4 + gg
                nc.vector.tensor_copy(hT[t][:, g * 128:(g + 1) * 128], tp[:])
                st["left"] -= 1
                if st["left"] == 0:
                    del ust[u]

            def s_phase(u, prev, fillers=None, prev_mmajor=False):
                """8 m-slots: S(u,m) + exp + prev's PV steps + filler work.

                prev's PV: acc A finishes by slot 3, its norms issue at slot 4
                (DVE latency hidden behind B's steps, so the acc ring never
                stalls the unit after); B finishes at slot 7, norms at end.
                prev_mmajor: step m at slot m for both tiles instead — u0's
                PV must wait for late v tiles that are still DMA-paced.
                Pending transposes (from the unit before prev) drop into even
                slots, a full unit after their norms — the Ldweights that
                loads h_sb never reaches PE.SEQ before its data is ready.
                Fillers keep per-slot PE work above the ScalarE exp pace so
                the 2-deep s_ps ring never throttles the pipeline.
                """
                nh, t = u
                fillers = fillers or {}
                ust[u] = {"p": [], "h": [None] * 4, "left": 4}
                for m in range(MT):
                    s_ps = ps_s.tile([128, 1024], F32, name="s_ps", tag="s")
                    for j in range(2):
                        psl = slice(j * 64, (j + 1) * 64)
                        nc.tensor.matmul(s_ps[:, j * 512:(j + 1) * 512],
                                         qkT[6 + t][psl, m * 128:(m + 1) * 128],
                                         qkT[t][psl, nh * 512:(nh + 1) * 512],
                                         start=True, stop=True)
                    p = pt_pool.tile([128, 1024], BF16, name="p_sb", tag="p")
                    nc.scalar.activation(p[:], s_ps[:], EXP, scale=SCALE)
                    if debug and u == (0, 0) and m == 0:
                        nc.sync.dma_start(dbg_p[:, :], p[:])
                    ust[u]["p"].append(p)
                    if prev is not None:
                        if prev_mmajor:
                            pv_steps(prev, "A", [m])
                            pv_steps(prev, "B", [m])
                        elif m < 4:
                            pv_steps(prev, "A", [2 * m, 2 * m + 1])
                        else:
                            pv_steps(prev, "B", [2 * (m - 4), 2 * (m - 4) + 1])
                        if m == 4 and not prev_mmajor:
                            norm_tile(prev, "A")
                    if m % 2 == 0 and pending_T:
                        transpose_g(*pending_T.pop(0))
                    for fn in fillers.get(m, ()):
                        fn()
                if prev is not None:
                    if debug and prev == (0, 0):
                        for nm, d_d in (("accA", dbg_accA), ("accB", dbg_accB)):
                            stg = stage_pool.tile([128, 512], F32, name="dbgs",
                                                  tag="wps")
                            nc.vector.tensor_copy(stg[:], ust[prev][nm][:])
                            nc.sync.dma_start(d_d[:, :], stg[:])
                    if prev_mmajor:
                        norm_tile(prev, "A")
                    norm_tile(prev, "B")
                    if debug and prev == (0, 0):
                        for gg in range(4):
                            nc.sync.dma_start(dbg_h[gg], ust[prev]["h"][gg][:])
                    for gg in range(4):
                        pending_T.append((prev, gg))

            # ---- phase 3: half a y tile; one contiguous DMA per full tile
            # (half-tile stores cost ~1-2.5us each on SP.SEQ descriptor gen)
            y_tiles = {}

            def proj_half(m, ph, dve_evict=True):
                ps = ps_a.tile([128, 384], F32, name="ps_y", tag="mm1")
                for k in range(KT):
                    nc.tensor.matmul(ps[:], hT[k][:, m * 128:(m + 1) * 128],
                                     wp[k][:, ph * 384:(ph + 1) * 384],
                                     start=(k == 0), stop=(k == KT - 1))
                if ph == 0:
                    y_tiles[m] = y_pool.tile([128, C], F32, name="y_sb", tag="y")
                y_sb = y_tiles[m]
                if dve_evict:
                    nc.vector.tensor_copy(y_sb[:, ph * 384:(ph + 1) * 384], ps[:])
                else:
                    nc.scalar.copy(y_sb[:, ph * 384:(ph + 1) * 384], ps[:])
                if ph == 1:
                    nc.sync.dma_start(y_d[m * 128:(m + 1) * 128, :], y_sb[:])

            def load_wp():
                for k in range(KT):
                    stg = stage_pool.tile([128, C], F32R, name="wps", tag="wps")
                    nc.sync.dma_start(stg[:], wproj_d[k * 128:(k + 1) * 128, :])
                    nc.vector.tensor_copy(wp[k][:], stg[:])

            # ---- emission schedule ----
            # Unit order: (0,0), (1,0) — the second costs no new weight DMA
            # (q0h1 computes from the already-loaded t0 slices), keeping the
            # exp stream alive while v tiles cook — then nh0's remaining t
            # (so n-half-0 proj can overlap late nh1 units), then nh1's.
            # Filler placement keeps every s_phase slot's PE work at or above
            # the ScalarE exp pace (EDF for qk halves: kT full + qT's working
            # half before a pair's first unit, the other qT half before the
            # pair's nh1 unit; v tiles late enough for their DMA; proj last).
            units = ([(0, 0), (1, 0)] + [(0, t) for t in range(1, 6)]
                     + [(1, t) for t in range(1, 6)])
            qk_headgroup([(0, 0), (6, 0)])                   # xt-h0 paced
            qk_headgroup([(6, 1), (0, 1)])                   # xt-h1 paced
            s_phase(units[0], None, {
                4: [lambda: v_half(0, 0), lambda: v_half(0, 1)],
                5: [lambda: v_half(1, 0), lambda: v_half(1, 1)],
                6: [lambda: v_half(2, 0), lambda: v_half(2, 1)],
                7: [lambda: v_half(3, 0), lambda: v_half(3, 1)]})
            F = {}
            F[1] = {0: [lambda: v_half(4, 0), lambda: v_half(4, 1)],
                    1: [lambda: v_half(5, 0), lambda: v_half(5, 1)],
                    2: [lambda: v_half(6, 0), lambda: v_half(6, 1)],
                    3: [lambda: v_half(7, 0), lambda: v_half(7, 1)],
                    4: [lambda: qk_half(1, 0)], 5: [lambda: qk_half(7, 0)],
                    6: [lambda: qk_half(7, 1)]}
            for i, tq in ((2, 2), (3, 3), (4, 4), (5, 5)):
                F[i] = {1: [lambda tq=tq: qk_half(tq, 0)],
                        3: [lambda tq=tq: qk_half(tq + 6, 0)],
                        5: [lambda tq=tq: qk_half(tq + 6, 1)]}
            F[6] = {1: [lambda: qk_half(1, 1)], 3: [load_wp]}
            F[7] = {1: [lambda: qk_half(2, 1)]}
            for i in (8, 9, 10):
                mp = i - 8
                F[i] = {1: [lambda i=i: qk_half(i - 5, 1)],
                        3: [lambda mp=mp: proj_half(mp, 0)],
                        5: [lambda mp=mp: proj_half(mp, 1)]}
            F[11] = {1: [lambda: proj_half(3, 0)], 3: [lambda: proj_half(3, 1)]}
            for i in range(1, 12):
                s_phase(units[i], units[i - 1], F.get(i), prev_mmajor=(i == 1))
            # tail: last unit's PV / norms / transposes / proj, pipelined
            # (transposes run one g ahead of proj so the hT evict's DVE
            # latency hides under the previous proj's matmuls)
            last = units[11]
            for m in range(4):
                pv_steps(last, "A", [2 * m, 2 * m + 1])
                if pending_T:                    # u10's deferred transposes
                    transpose_g(*pending_T.pop(0))
            norm_tile(last, "A")
            for m in range(4):
                pv_steps(last, "B", [2 * m, 2 * m + 1])
                if pending_T:
                    transpose_g(*pending_T.pop(0))
            norm_tile(last, "B")
            transpose_g(last, 0)
            transpose_g(last, 1)
            for gg in range(4):
                if gg < 2:
                    transpose_g(last, gg + 2)
                proj_half(4 + gg, 0, dve_evict=False)
                proj_half(4 + gg, 1, dve_evict=False)
            if debug:
                for t in range(12):
                    nc.sync.dma_start(dbg_qkT[t], qkT[t][:])
                for m in range(MT):
                    nc.sync.dma_start(dbg_v[m], v_bf[m][:])
                for k in range(KT):
                    nc.sync.dma_start(dbg_hT[k], hT[k][:])

    nc.compile()
    return nc


def _run(inputs, trace=False, trace_kwargs=None):
    global _cached_nc
    x = np.asarray(inputs["x"], dtype=np.float32)
    wqkv = np.ascontiguousarray(np.asarray(inputs["W_qkv"], dtype=np.float32))
    wproj = np.ascontiguousarray(np.asarray(inputs["W_proj"], dtype=np.float32))
    xT = np.ascontiguousarray(x.transpose(0, 2, 1))  # [B, C, N]

    if _cached_nc is None:
        _cached_nc = build_program()
    nc = _cached_nc

    in_maps = [{"xT": xT[b], "wqkv": wqkv, "wproj": wproj} for b in range(B)]
    kwargs = {}
    if trace:
        kwargs["trace"] = True
        if trace_kwargs:
            kwargs.update(trace_kwargs)
    try:
        res = run_bass_kernel_spmd(nc, in_maps, core_ids=list(range(B)), **kwargs)
    except Exception:
        # transient axon/PJRT hiccups happen; one retry
        res = run_bass_kernel_spmd(nc, in_maps, core_ids=list(range(B)), **kwargs)
    out = np.stack([r["y"] for r in res.results], axis=0)
    return out, res


def kernel(**inputs):
    out, _ = _run(inputs)
    return out


# revision 56
# speedup vs baseline: 1.2468x; 1.0039x over previous
"""Multi-head attention (B=8, N=1024, H=12, D=64, C=768) on 8 trn2 cores.

Sharding: data-parallel over batch. Core b computes attention for x[b];
weights are replicated. No collectives.

Per-core dataflow:
  qkT[12][128,1024] bf16 : d-major Q^T/K^T   (f32r matmul, bf16 evict)
  v_bf[8][128,12,65] bf16: n-major V per m-tile + ones column (rowsum)
  per unit u=(nh,t) over 8 m-slots:
     S^T[m,n] = k^T.T @ q^T  (bf16, psum f32, 2 heads x 512 n)
     P^T = exp(S^T/8)        (ScalarE, bf16 out)
     prev unit's PV chunk:   acc[128 n, 65] += P^T-slice.T @ v_aug
                             (bf16 operands, 128 n-partitions: half the
                              PE cycles of the d-major form)
     prev norms:             recip(rowsum col) + per-partition scale (DVE)
  h[128 n, 128 dpair] --PE transpose--> hT[t][128, 1024] bf16
  y = hT.T @ W_proj (bf16) ; proj of n-half 0 overlaps nh=1 attention.
"""
import numpy as np

import concourse.bass as bass
import concourse.mybir as mybir
import concourse.tile as tile
from concourse import bacc
from concourse.bass_utils import run_bass_kernel_spmd
from concourse.masks import make_identity

F32R = mybir.dt.float32r
F32 = mybir.dt.float32
BF16 = mybir.dt.bfloat16
EXP = mybir.ActivationFunctionType.Exp

B, N, C = 8, 1024, 768
H, D = 12, 64
HID = H * D  # 768
KT = C // 128          # 6 feature k-tiles
MT = N // 128          # 8 sequence m-tiles
SCALE = D ** -0.5      # 0.125

_cached_nc = None


def build_program(debug=False):
    nc = bacc.Bacc(None, target_bir_lowering=False)

    xT_d = nc.dram_tensor("xT", [C, N], F32R, kind="ExternalInput")
    wqkv_d = nc.dram_tensor("wqkv", [C, 3 * HID], F32R, kind="ExternalInput")
    wproj_d = nc.dram_tensor("wproj", [HID, C], F32R, kind="ExternalInput")
    y_d = nc.dram_tensor("y", [N, C], F32, kind="ExternalOutput")
    if debug:
        dbg_qkT = nc.dram_tensor("dbg_qkT", [12, 128, N], BF16, kind="ExternalOutput")
        dbg_v = nc.dram_tensor("dbg_v", [MT, 128, H, D + 1], BF16,
                               kind="ExternalOutput")
        dbg_hT = nc.dram_tensor("dbg_hT", [KT, 128, N], BF16, kind="ExternalOutput")
        dbg_p = nc.dram_tensor("dbg_p", [128, 1024], BF16, kind="ExternalOutput")
        dbg_accA = nc.dram_tensor("dbg_accA", [128, 512], F32, kind="ExternalOutput")
        dbg_accB = nc.dram_tensor("dbg_accB", [128, 512], F32, kind="ExternalOutput")
        dbg_h = nc.dram_tensor("dbg_h", [4, 128, 128], BF16, kind="ExternalOutput")

    with tile.TileContext(nc) as tc:
        with tc.tile_pool(name="persist", bufs=1) as persist, \
             tc.tile_pool(name="pt_pool", bufs=18) as pt_pool, \
             tc.tile_pool(name="hsb_pool", bufs=12) as hsb_pool, \
             tc.tile_pool(name="nrm_pool", bufs=8) as nrm_pool, \
             tc.tile_pool(name="stage_pool", bufs=2) as stage_pool, \
             tc.tile_pool(name="y_pool", bufs=3) as y_pool, \
             tc.tile_pool(name="ps_a", bufs=2, space="PSUM") as ps_a, \
             tc.tile_pool(name="ps_s", bufs=2, space="PSUM") as ps_s, \
             tc.tile_pool(name="ps_acc", bufs=2, space="PSUM") as ps_acc:

            # ---- resident tiles ----
            xt = [persist.tile([128, N], F32R, name=f"xt{k}", tag=f"xt{k}")
                  for k in range(KT)]
            wqk = [persist.tile([128, 2 * HID], F32R, name=f"wqk{k}", tag=f"wqk{k}")
                   for k in range(KT)]
            wv = [persist.tile([128, HID], F32R, name=f"wv{k}", tag=f"wv{k}")
                  for k in range(KT)]
            wp = [persist.tile([128, C], BF16, name=f"wp{k}", tag=f"wp{k}")
                  for k in range(KT)]
            qkT = [persist.tile([128, N], BF16, name=f"qkT{t}", tag=f"qkT{t}")
                   for t in range(12)]
            v_bf = [persist.tile([128, H, D + 1], BF16, name=f"vbf{m}", tag=f"vbf{m}")
                    for m in range(MT)]
            hT = [persist.tile([128, N], BF16, name=f"hT{t}", tag=f"hT{t}")
                  for t in range(KT)]
            ident = persist.tile([128, 128], BF16, name="ident", tag="ident")

            # ---- DMA emission order = priority ----
            # qk_half(0,0)/(6,*) need the t0/t6 wqk column slices + x; x's
            # n-halves split so the first S chain starts off xt-h0 alone.
            # wv lands in time for v tiles in u0's late slots; t1/t7 slices
            # before the bulk so unit (0,1) can start; rest streams after.
            for k in range(KT):
                nc.sync.dma_start(xt[k][:, 0:512],
                                  xT_d[k * 128:(k + 1) * 128, 0:512])
                for c0 in (0, 768):              # t0, t6 col slices
                    nc.sync.dma_start(wqk[k][:, c0:c0 + 128],
                                      wqkv_d[k * 128:(k + 1) * 128, c0:c0 + 128])
            for k in range(KT):
                nc.sync.dma_start(xt[k][:, 512:1024],
                                  xT_d[k * 128:(k + 1) * 128, 512:1024])
            for k in range(KT):
                nc.sync.dma_start(wv[k][:], wqkv_d[k * 128:(k + 1) * 128, 2 * HID:])
            for k in range(KT):
                for c0 in (128, 896):            # t1, t7 col slices
                    nc.sync.dma_start(wqk[k][:, c0:c0 + 128],
                                      wqkv_d[k * 128:(k + 1) * 128, c0:c0 + 128])
            for k in range(KT):
                nc.sync.dma_start(wqk[k][:, 256:768],
                                  wqkv_d[k * 128:(k + 1) * 128, 256:768])
                nc.sync.dma_start(wqk[k][:, 1024:1536],
                                  wqkv_d[k * 128:(k + 1) * 128, 1024:1536])

            # warm the exp table during the DMA prefix
            warm = persist.tile([1, 8], F32, name="warm", tag="warm")
            nc.gpsimd.memset(warm[:], 0.0)
            nc.scalar.activation(warm[:], warm[:], EXP)
            make_identity(nc, ident[:])

            # ---- phase 1a: half a qkT tile (bf16 evict) ----
            def qk_half(t, nh):
                ps = ps_a.tile([128, 512], F32, name="ps_qk", tag="mm1")
                for k in range(KT):
                    nc.tensor.matmul(ps[:], wqk[k][:, t * 128:(t + 1) * 128],
                                     xt[k][:, nh * 512:(nh + 1) * 512],
                                     start=(k == 0), stop=(k == KT - 1))
                nc.vector.tensor_copy(qkT[t][:, nh * 512:(nh + 1) * 512], ps[:])

            def qk_headgroup(specs):
                """k-step-major interleave of several qk chains, so each
                arriving xt k-tile immediately feeds every chain (the head
                is DMA-paced; chain-major would idle PE between k-tiles).
                ps_s is idle this early — borrow it for the extra chains.
                """
                states = []
                for ci, (t, nh) in enumerate(specs):
                    if ci < 2:
                        ps = ps_a.tile([128, 512], F32, name="ps_qk", tag="mm1")
                    else:
                        ps = ps_s.tile([128, 1024], F32, name="s_ps",
                                       tag="s")[:, 0:512]
                    states.append((t, nh, ps))
                for k in range(KT):
                    for t, nh, ps in states:
                        nc.tensor.matmul(ps, wqk[k][:, t * 128:(t + 1) * 128],
                                         xt[k][:, nh * 512:(nh + 1) * 512],
                                         start=(k == 0), stop=(k == KT - 1))
                for t, nh, ps in states:
                    nc.vector.tensor_copy(qkT[t][:, nh * 512:(nh + 1) * 512], ps)

            # ---- phase 1b: v half-tiles (n-major, bf16, ones col) ----
            def v_half(m, vh):
                ps = ps_a.tile([128, 384], F32, name="ps_v", tag="mm1")
                for k in range(KT):
                    nc.tensor.matmul(ps[:], xt[k][:, m * 128:(m + 1) * 128],
                                     wv[k][:, vh * 384:(vh + 1) * 384],
                                     start=(k == 0), stop=(k == KT - 1))
                dst = v_bf[m][:, vh * 6:(vh + 1) * 6, 0:D]
                nc.vector.tensor_copy(dst, ps[:].rearrange("p (h d) -> p h d", d=D))
                if vh == 1:
                    nc.gpsimd.memset(v_bf[m][:, :, D:D + 1], 1.0)

            # ---- per-unit state ----
            ust = {}

            def pv_steps(u, half, ms):
                """PV accumulation steps `ms` for acc tile A (regions 0-3) or
                B (4-7) of unit u. Region c: gg=c//2 (n-subtile), j=c%2 (head
                in pair); regions live as 65-col strips at 128-col offsets.
                All of u's p tiles already exist when its PV runs (one unit
                later), so A can finish early — its norms then hide their DVE
                latency behind B's steps, and acc-tile ring reuse never
                stalls the next unit.
                """
                nh, t = u
                st = ust[u]
                key = "acc" + half
                if key not in st:
                    st[key] = ps_acc.tile([128, 512], F32, name="acc", tag="acc")
                acc = st[key]
                cs = range(4) if half == "A" else range(4, 8)
                for m in ms:
                    p = st["p"][m]
                    for c in cs:
                        gg, j = c // 2, c % 2
                        col = (c % 4) * 128
                        # start=True clears has_written for the whole psum
                        # bank row, so only the tile's first region may set
                        # it; the other regions' first step lands on cleared
                        # has_written and overwrites.
                        nc.tensor.matmul(acc[:, col:col + D + 1],
                                         p[:, j * 512 + gg * 128:
                                           j * 512 + (gg + 1) * 128],
                                         v_bf[m][:, 2 * t + j, :],
                                         start=(m == 0 and c % 4 == 0),
                                         stop=(m == MT - 1),
                                         skip_group_check=True)

            def norm_tile(u, half):
                """1/rowsum (col 64 of regions) * out -> h_sb[gg][:, j*64:]"""
                st = ust[u]
                acc = st["acc" + half]
                rs = nrm_pool.tile([128, 4], F32, name="rs", tag="rs")
                nc.vector.reciprocal(
                    rs[:], acc[:].rearrange("p (g c) -> p g c", c=128)[:, :, D])
                for ci in range(4):
                    c = ci if half == "A" else ci + 4
                    gg, j = c // 2, c % 2
                    col = ci * 128
                    if j == 0:
                        st["h"][gg] = hsb_pool.tile([128, 128], BF16,
                                                    name="h_sb", tag="h")
                    nc.vector.tensor_scalar_mul(st["h"][gg][:, j * 64:(j + 1) * 64],
                                                acc[:, col:col + D],
                                                rs[:, ci:ci + 1])

            pending_T = []   # (unit, gg) transposes deferred ~1 unit for slack

            def s_slot(u, m):
                nh, t = u
                s_ps = ps_s.tile([128, 1024], F32, name="s_ps", tag="s")
                for j in range(2):
                    psl = slice(j * 64, (j + 1) * 64)
                    nc.tensor.matmul(s_ps[:, j * 512:(j + 1) * 512],
                                     qkT[6 + t][psl, m * 128:(m + 1) * 128],
                                     qkT[t][psl, nh * 512:(nh + 1) * 512],
                                     start=True, stop=True)
                p = pt_pool.tile([128, 1024], BF16, name="p_sb", tag="p")
                nc.scalar.activation(p[:], s_ps[:], EXP, scale=SCALE)
                ust[u]["p"].append(p)

            def transpose_g(u, gg):
                nh, t = u
                st = ust[u]
                tp = ps_a.tile([128, 128], BF16, name="tp", tag="mm1")
                nc.tensor.transpose(tp[:], st["h"][gg][:], ident[:])
                g = nh * 4 + gg
                nc.vector.tensor_copy(hT[t][:, g * 128:(g + 1) * 128], tp[:])
                st["left"] -= 1
                if st["left"] == 0:
                    del ust[u]

            def s_phase(u, prev, fillers=None, prev_mmajor=False):
                """8 m-slots: S(u,m) + exp + prev's PV steps + filler work.

                prev's PV: acc A finishes by slot 3, its norms issue at slot 4
                (DVE latency hidden behind B's steps, so the acc ring never
                stalls the unit after); B finishes at slot 7, norms at end.
                prev_mmajor: step m at slot m for both tiles instead — u0's
                PV must wait for late v tiles that are still DMA-paced.
                Pending transposes (from the unit before prev) drop into even
                slots, a full unit after their norms — the Ldweights that
                loads h_sb never reaches PE.SEQ before its data is ready.
                Fillers keep per-slot PE work above the ScalarE exp pace so
                the 2-deep s_ps ring never throttles the pipeline.
                """
                nh, t = u
                fillers = fillers or {}
                if u not in ust:
                    ust[u] = {"p": [], "h": [None] * 4, "left": 4}
                for m in range(len(ust[u]["p"]), MT):
                    s_slot(u, m)
                    if debug and u == (0, 0) and m == 0:
                        nc.sync.dma_start(dbg_p[:, :], ust[u]["p"][m][:])
                    if prev is not None:
                        if prev_mmajor:
                            pv_steps(prev, "A", [m])
                            pv_steps(prev, "B", [m])
                        elif m < 4:
                            pv_steps(prev, "A", [2 * m, 2 * m + 1])
                        else:
                            pv_steps(prev, "B", [2 * (m - 4), 2 * (m - 4) + 1])
                        if m == 4 and not prev_mmajor:
                            norm_tile(prev, "A")
                    if m % 2 == 0 and pending_T:
                        transpose_g(*pending_T.pop(0))
                    for fn in fillers.get(m, ()):
                        fn()
                if prev is not None:
                    if debug and prev == (0, 0):
                        for nm, d_d in (("accA", dbg_accA), ("accB", dbg_accB)):
                            stg = stage_pool.tile([128, 512], F32, name="dbgs",
                                                  tag="wps")
                            nc.vector.tensor_copy(stg[:], ust[prev][nm][:])
                            nc.sync.dma_start(d_d[:, :], stg[:])
                    if prev_mmajor:
                        norm_tile(prev, "A")
                    norm_tile(prev, "B")
                    if debug and prev == (0, 0):
                        for gg in range(4):
                            nc.sync.dma_start(dbg_h[gg], ust[prev]["h"][gg][:])
                    for gg in range(4):
                        pending_T.append((prev, gg))

            # ---- phase 3: half a y tile; one contiguous DMA per full tile
            # (half-tile stores cost ~1-2.5us each on SP.SEQ descriptor gen)
            y_tiles = {}

            def proj_half(m, ph, dve_evict=True, final=False):
                ps = ps_a.tile([128, 384], F32, name="ps_y", tag="mm1")
                for k in range(KT):
                    nc.tensor.matmul(ps[:], hT[k][:, m * 128:(m + 1) * 128],
                                     wp[k][:, ph * 384:(ph + 1) * 384],
                                     start=(k == 0), stop=(k == KT - 1))
                if ph == 0:
                    y_tiles[m] = y_pool.tile([128, C], F32, name="y_sb", tag="y")
                y_sb = y_tiles[m]
                if dve_evict:
                    nc.vector.tensor_copy(y_sb[:, ph * 384:(ph + 1) * 384], ps[:])
                else:
                    nc.scalar.copy(y_sb[:, ph * 384:(ph + 1) * 384], ps[:])
                if final:   # last tile: per-half stores shorten the exit path
                    nc.sync.dma_start(
                        y_d[m * 128:(m + 1) * 128, ph * 384:(ph + 1) * 384],
                        y_sb[:, ph * 384:(ph + 1) * 384])
                elif ph == 1:
                    nc.sync.dma_start(y_d[m * 128:(m + 1) * 128, :], y_sb[:])

            def load_wp():
                for k in range(KT):
                    stg = stage_pool.tile([128, C], F32R, name="wps", tag="wps")
                    nc.sync.dma_start(stg[:], wproj_d[k * 128:(k + 1) * 128, :])
                    nc.vector.tensor_copy(wp[k][:], stg[:])

            # ---- emission schedule ----
            # Unit order: (0,0), (1,0) — the second costs no new weight DMA
            # (q0h1 computes from the already-loaded t0 slices), keeping the
            # exp stream alive while v tiles cook — then nh0's remaining t
            # (so n-half-0 proj can overlap late nh1 units), then nh1's.
            # Filler placement keeps every s_phase slot's PE work at or above
            # the ScalarE exp pace (EDF for qk halves: kT full + qT's working
            # half before a pair's first unit, the other qT half before the
            # pair's nh1 unit; v tiles late enough for their DMA; proj last).
            units = ([(0, 0), (1, 0)] + [(0, t) for t in range(1, 6)]
                     + [(1, t) for t in range(1, 6)])
            qk_headgroup([(0, 0), (6, 0)])                   # xt-h0 paced
            qk_headgroup([(6, 1), (0, 1)])                   # xt-h1 paced
            s_phase(units[0], None, {
                4: [lambda: v_half(0, 0), lambda: v_half(0, 1)],
                5: [lambda: v_half(1, 0), lambda: v_half(1, 1)],
                6: [lambda: v_half(2, 0), lambda: v_half(2, 1)],
                7: [lambda: v_half(3, 0), lambda: v_half(3, 1)]})
            F = {}
            F[1] = {0: [lambda: v_half(4, 0), lambda: v_half(4, 1)],
                    1: [lambda: v_half(5, 0), lambda: v_half(5, 1)],
                    2: [lambda: v_half(6, 0), lambda: v_half(6, 1)],
                    3: [lambda: v_half(7, 0), lambda: v_half(7, 1)],
                    4: [lambda: qk_half(1, 0)], 5: [lambda: qk_half(7, 0)],
                    6: [lambda: qk_half(7, 1)]}
            for i, tq in ((2, 2), (3, 3), (4, 4), (5, 5)):
                F[i] = {1: [lambda tq=tq: qk_half(tq, 0)],
                        3: [lambda tq=tq: qk_half(tq + 6, 0)],
                        5: [lambda tq=tq: qk_half(tq + 6, 1)]}
            F[6] = {1: [lambda: qk_half(1, 1)], 3: [load_wp]}
            F[7] = {1: [lambda: qk_half(2, 1)]}
            for i in (8, 9, 10):
                mp = i - 8
                F[i] = {1: [lambda i=i: qk_half(i - 5, 1)],
                        3: [lambda mp=mp: proj_half(mp, 0)],
                        5: [lambda mp=mp: proj_half(mp, 1)]}
            F[11] = {1: [lambda: proj_half(3, 0)], 3: [lambda: proj_half(3, 1)]}
            for i in range(1, 12):
                s_phase(units[i], units[i - 1], F.get(i), prev_mmajor=(i == 1))
            # tail: last unit's PV / norms / transposes / proj, pipelined
            # (transposes run one g ahead of proj so the hT evict's DVE
            # latency hides under the previous proj's matmuls)
            last = units[11]
            for m in range(4):
                pv_steps(last, "A", [2 * m, 2 * m + 1])
                if pending_T:                    # u10's deferred transposes
                    transpose_g(*pending_T.pop(0))
            norm_tile(last, "A")
            for m in range(4):
                pv_steps(last, "B", [2 * m, 2 * m + 1])
                if pending_T:
                    transpose_g(*pending_T.pop(0))
            norm_tile(last, "B")
            transpose_g(last, 0)
            transpose_g(last, 1)
            for gg in range(4):
                if gg < 2:
                    transpose_g(last, gg + 2)
                proj_half(4 + gg, 0, dve_evict=False, final=(gg == 3))
                proj_half(4 + gg, 1, dve_evict=False, final=(gg == 3))
            if debug:
                for t in range(12):
                    nc.sync.dma_start(dbg_qkT[t], qkT[t][:])
                for m in range(MT):
                    nc.sync.dma_start(dbg_v[m], v_bf[m][:])
                for k in range(KT):
                    nc.sync.dma_start(dbg_hT[k], hT[k][:])

    nc.compile()
    return nc


def _run(inputs, trace=False, trace_kwargs=None):
    global _cached_nc
    x = np.asarray(inputs["x"], dtype=np.float32)
    wqkv = np.ascontiguousarray(np.asarray(inputs["W_qkv"], dtype=np.float32))
    wproj = np.ascontiguousarray(np.asarray(inputs["W_proj"], dtype=np.float32))
    xT = np.ascontiguousarray(x.transpose(0, 2, 1))  # [B, C, N]

    if _cached_nc is None:
        _cached_nc = build_program()
    nc = _cached_nc

    in_maps = [{"xT": xT[b], "wqkv": wqkv, "wproj": wproj} for b in range(B)]
    kwargs = {}
    if trace:
        kwargs["trace"] = True
        if trace_kwargs:
            kwargs.update(trace_kwargs)
    try:
        res = run_bass_kernel_spmd(nc, in_maps, core_ids=list(range(B)), **kwargs)
    except Exception:
        # transient axon/PJRT hiccups happen; one retry
        res = run_bass_kernel_spmd(nc, in_maps, core_ids=list(range(B)), **kwargs)
    out = np.stack([r["y"] for r in res.results], axis=0)
    return out, res


def kernel(**inputs):
    out, _ = _run(inputs)
    return out


# revision 73
# speedup vs baseline: 1.2866x; 1.0319x over previous
"""Multi-head attention (B=8, N=1024, H=12, D=64, C=768) on 8 trn2 cores.

Sharding: data-parallel over batch. Core b computes attention for x[b];
weights are replicated. No collectives.

Per-core dataflow:
  qkT[12][128,1024] bf16 : d-major Q^T/K^T   (f32r matmul, bf16 evict)
  v_bf[8][128,12,65] bf16: n-major V per m-tile + ones column (rowsum)
  per unit u=(nh,t) over 8 m-slots:
     S^T[m,n] = k^T.T @ q^T  (bf16, psum f32, 2 heads x 512 n)
     P^T = exp(S^T/8)        (ScalarE, bf16 out)
     prev unit's PV chunk:   acc[128 n, 65] += P^T-slice.T @ v_aug
                             (bf16 operands, 128 n-partitions: half the
                              PE cycles of the d-major form)
     prev norms:             recip(rowsum col) + per-partition scale (DVE)
  h[128 n, 128 dpair] --PE transpose--> hT[t][128, 1024] bf16
  y = hT.T @ W_proj (bf16) ; proj of n-half 0 overlaps nh=1 attention.
"""
import numpy as np

import concourse.bass as bass
import concourse.mybir as mybir
import concourse.tile as tile
from concourse import bacc
from concourse.bass_utils import run_bass_kernel_spmd
from concourse.masks import make_identity

F32R = mybir.dt.float32r
F32 = mybir.dt.float32
BF16 = mybir.dt.bfloat16
EXP = mybir.ActivationFunctionType.Exp

B, N, C = 8, 1024, 768
H, D = 12, 64
HID = H * D  # 768
KT = C // 128          # 6 feature k-tiles
MT = N // 128          # 8 sequence m-tiles
SCALE = D ** -0.5      # 0.125

_cached_nc = None


def build_program(debug=False):
    nc = bacc.Bacc(None, target_bir_lowering=False)

    xT_d = nc.dram_tensor("xT", [C, N], F32R, kind="ExternalInput")
    wqkv_d = nc.dram_tensor("wqkv", [C, 3 * HID], F32R, kind="ExternalInput")
    wproj_d = nc.dram_tensor("wproj", [HID, C], F32R, kind="ExternalInput")
    y_d = nc.dram_tensor("y", [N, C], F32, kind="ExternalOutput")
    if debug:
        dbg_qkT = nc.dram_tensor("dbg_qkT", [12, 128, N], BF16, kind="ExternalOutput")
        dbg_v = nc.dram_tensor("dbg_v", [MT, 128, H, D + 1], BF16,
                               kind="ExternalOutput")
        dbg_hT = nc.dram_tensor("dbg_hT", [KT, 128, N], BF16, kind="ExternalOutput")
        dbg_p = nc.dram_tensor("dbg_p", [128, 1024], BF16, kind="ExternalOutput")
        dbg_accA = nc.dram_tensor("dbg_accA", [128, 512], F32, kind="ExternalOutput")
        dbg_accB = nc.dram_tensor("dbg_accB", [128, 512], F32, kind="ExternalOutput")
        dbg_h = nc.dram_tensor("dbg_h", [4, 128, 128], BF16, kind="ExternalOutput")

    with tile.TileContext(nc) as tc:
        with tc.tile_pool(name="persist", bufs=1) as persist, \
             tc.tile_pool(name="pt_pool", bufs=18) as pt_pool, \
             tc.tile_pool(name="hsb_pool", bufs=12) as hsb_pool, \
             tc.tile_pool(name="nrm_pool", bufs=8) as nrm_pool, \
             tc.tile_pool(name="stage_pool", bufs=2) as stage_pool, \
             tc.tile_pool(name="y_pool", bufs=3) as y_pool, \
             tc.tile_pool(name="ps_a", bufs=2, space="PSUM") as ps_a, \
             tc.tile_pool(name="ps_s", bufs=2, space="PSUM") as ps_s, \
             tc.tile_pool(name="ps_acc", bufs=2, space="PSUM") as ps_acc:

            # ---- resident tiles ----
            # x and wv live in single tiles (k-tiles along free dim) so one
            # chunk DMA can carry several k-tiles — per-DMA SP-issue + HWDGE
            # overhead (~1.3us serial each) dominates small head transfers
            xt_all = persist.tile([128, KT, N], F32R, name="xt", tag="xt")
            xt = [xt_all[:, k, :] for k in range(KT)]
            wqk = [persist.tile([128, 2 * HID], F32R, name=f"wqk{k}", tag=f"wqk{k}")
                   for k in range(KT)]
            wv_all = persist.tile([128, KT, HID], F32R, name="wv", tag="wv")
            wv = [wv_all[:, k, :] for k in range(KT)]
            wp = [persist.tile([128, C], BF16, name=f"wp{k}", tag=f"wp{k}")
                  for k in range(KT)]
            qkT = [persist.tile([128, N], BF16, name=f"qkT{t}", tag=f"qkT{t}")
                   for t in range(12)]
            v_bf = [persist.tile([128, H, D + 1], BF16, name=f"vbf{m}", tag=f"vbf{m}")
                    for m in range(MT)]
            hT = [persist.tile([128, N], BF16, name=f"hT{t}", tag=f"hT{t}")
                  for t in range(KT)]
            ident = persist.tile([128, 128], BF16, name="ident", tag="ident")

            # ---- DMA emission order = priority ----
            # qk_half(0,0)/(6,*) need the t0/t6 wqk column slices + x; x's
            # n-halves split so the first S chain starts off xt-h0 alone.
            # wv lands in time for v tiles in u0's late slots; t1/t7 slices
            # before the bulk so unit (0,1) can start; rest streams after.
            def slice_pair(k, c0):
                """One strided DMA for wqk cols [c0:c0+128] U [c0+768:+128]
                (the q and k slices of one head pair) — a separate DMA per
                slice costs more SP-issue + HWDGE time than the transfer."""
                rows = slice(k * 128, (k + 1) * 128)
                src = wqkv_d[rows, c0:c0 + 896].rearrange(
                    "p (b c) -> p b c", c=128)[:, 0:7:6, :]
                dst = wqk[k][:, c0:c0 + 896].rearrange(
                    "p (b c) -> p b c", c=128)[:, 0:7:6, :]
                nc.sync.dma_start(dst, src)

            def x_chunk(k0, k1, nsl):
                src = xT_d[k0 * 128:k1 * 128, nsl].rearrange(
                    "(kk p) n -> p kk n", p=128)
                nc.sync.dma_start(xt_all[:, k0:k1, nsl], src)

            # x h0 in graduated chunks (first k-tile alone so the first
            # matmul isn't gated on a big transfer) + t0/t6 slices
            for k0, k1 in ((0, 1), (1, 3), (3, 6)):
                x_chunk(k0, k1, slice(0, 512))
                for k in range(k0, k1):
                    slice_pair(k, 0)
            for k0, k1 in ((0, 1), (1, 3), (3, 6)):     # x h1
                x_chunk(k0, k1, slice(512, 1024))
            for k0, k1 in ((0, 2), (2, 4), (4, 6)):      # wv
                src = wqkv_d[k0 * 128:k1 * 128, 2 * HID:].rearrange(
                    "(kk p) c -> p kk c", p=128)
                nc.sync.dma_start(wv_all[:, k0:k1, :], src)
            for k in range(KT):
                slice_pair(k, 128)               # t1 + t7 col slices
            for k in range(KT):
                nc.sync.dma_start(wqk[k][:, 256:768],
                                  wqkv_d[k * 128:(k + 1) * 128, 256:768])
                nc.sync.dma_start(wqk[k][:, 1024:1536],
                                  wqkv_d[k * 128:(k + 1) * 128, 1024:1536])

            # warm the exp table during the DMA prefix
            warm = persist.tile([1, 8], F32, name="warm", tag="warm")
            nc.gpsimd.memset(warm[:], 0.0)
            nc.scalar.activation(warm[:], warm[:], EXP)
            make_identity(nc, ident[:])

            # ---- phase 1a: half a qkT tile (bf16 evict) ----
            def qk_half(t, nh):
                ps = ps_a.tile([128, 512], F32, name="ps_qk", tag="mm1")
                for k in range(KT):
                    nc.tensor.matmul(ps[:], wqk[k][:, t * 128:(t + 1) * 128],
                                     xt[k][:, nh * 512:(nh + 1) * 512],
                                     start=(k == 0), stop=(k == KT - 1))
                nc.vector.tensor_copy(qkT[t][:, nh * 512:(nh + 1) * 512], ps[:])

            def qk_headgroup(specs):
                """k-step-major interleave of several qk chains, so each
                arriving xt k-tile immediately feeds every chain (the head
                is DMA-paced; chain-major would idle PE between k-tiles).
                ps_s is idle this early — borrow it for the extra chains.
                """
                states = []
                for ci, (t, nh) in enumerate(specs):
                    if ci < 2:
                        ps = ps_a.tile([128, 512], F32, name="ps_qk", tag="mm1")
                    else:
                        ps = ps_s.tile([128, 1024], F32, name="s_ps",
                                       tag="s")[:, 0:512]
                    states.append((t, nh, ps))
                for k in range(KT):
                    for t, nh, ps in states:
                        nc.tensor.matmul(ps, wqk[k][:, t * 128:(t + 1) * 128],
                                         xt[k][:, nh * 512:(nh + 1) * 512],
                                         start=(k == 0), stop=(k == KT - 1))
                for t, nh, ps in states:
                    nc.vector.tensor_copy(qkT[t][:, nh * 512:(nh + 1) * 512], ps)

            # ---- phase 1b: v half-tiles (n-major, bf16, ones col) ----
            def v_half(m, vh):
                ps = ps_a.tile([128, 384], F32, name="ps_v", tag="mm1")
                for k in range(KT):
                    nc.tensor.matmul(ps[:], xt[k][:, m * 128:(m + 1) * 128],
                                     wv[k][:, vh * 384:(vh + 1) * 384],
                                     start=(k == 0), stop=(k == KT - 1))
                dst = v_bf[m][:, vh * 6:(vh + 1) * 6, 0:D]
                nc.vector.tensor_copy(dst, ps[:].rearrange("p (h d) -> p h d", d=D))
                if vh == 1:
                    nc.gpsimd.memset(v_bf[m][:, :, D:D + 1], 1.0)

            # ---- per-unit state ----
            ust = {}

            def pv_steps(u, half, ms):
                """PV accumulation steps `ms` for acc tile A (regions 0-3) or
                B (4-7) of unit u. Region c: gg=c//2 (n-subtile), j=c%2 (head
                in pair); regions live as 65-col strips at 128-col offsets.
                All of u's p tiles already exist when its PV runs (one unit
                later), so A can finish early — its norms then hide their DVE
                latency behind B's steps, and acc-tile ring reuse never
                stalls the next unit.
                """
                nh, t = u
                st = ust[u]
                key = "acc" + half
                if key not in st:
                    st[key] = ps_acc.tile([128, 512], F32, name="acc", tag="acc")
                acc = st[key]
                cs = range(4) if half == "A" else range(4, 8)
                for m in ms:
                    p = st["p"][m]
                    for c in cs:
                        gg, j = c // 2, c % 2
                        col = (c % 4) * 128
                        # start=True clears has_written for the whole psum
                        # bank row, so only the tile's first region may set
                        # it; the other regions' first step lands on cleared
                        # has_written and overwrites.
                        nc.tensor.matmul(acc[:, col:col + D + 1],
                                         p[:, j * 512 + gg * 128:
                                           j * 512 + (gg + 1) * 128],
                                         v_bf[m][:, 2 * t + j, :],
                                         start=(m == 0 and c % 4 == 0),
                                         stop=(m == MT - 1),
                                         skip_group_check=True)

            def norm_tile(u, half):
                """1/rowsum (col 64 of regions) * out -> h_sb[gg][:, j*64:]"""
                st = ust[u]
                acc = st["acc" + half]
                rs = nrm_pool.tile([128, 4], F32, name="rs", tag="rs")
                nc.vector.reciprocal(
                    rs[:], acc[:].rearrange("p (g c) -> p g c", c=128)[:, :, D])
                for ci in range(4):
                    c = ci if half == "A" else ci + 4
                    gg, j = c // 2, c % 2
                    col = ci * 128
                    if j == 0:
                        st["h"][gg] = hsb_pool.tile([128, 128], BF16,
                                                    name="h_sb", tag="h")
                    nc.vector.tensor_scalar_mul(st["h"][gg][:, j * 64:(j + 1) * 64],
                                                acc[:, col:col + D],
                                                rs[:, ci:ci + 1])

            pending_T = []   # (unit, gg) transposes deferred ~1 unit for slack

            def s_slot(u, m):
                nh, t = u
                s_ps = ps_s.tile([128, 1024], F32, name="s_ps", tag="s")
                for j in range(2):
                    psl = slice(j * 64, (j + 1) * 64)
                    nc.tensor.matmul(s_ps[:, j * 512:(j + 1) * 512],
                                     qkT[6 + t][psl, m * 128:(m + 1) * 128],
                                     qkT[t][psl, nh * 512:(nh + 1) * 512],
                                     start=True, stop=True)
                p = pt_pool.tile([128, 1024], BF16, name="p_sb", tag="p")
                nc.scalar.activation(p[:], s_ps[:], EXP, scale=SCALE)
                ust[u]["p"].append(p)

            def transpose_g(u, gg):
                nh, t = u
                st = ust[u]
                tp = ps_a.tile([128, 128], BF16, name="tp", tag="mm1")
                nc.tensor.transpose(tp[:], st["h"][gg][:], ident[:])
                g = nh * 4 + gg
                nc.vector.tensor_copy(hT[t][:, g * 128:(g + 1) * 128], tp[:])
                st["left"] -= 1
                if st["left"] == 0:
                    del ust[u]

            def s_phase(u, prev, fillers=None, prev_mmajor=False):
                """8 m-slots: S(u,m) + exp + prev's PV steps + filler work.

                prev's PV: acc A finishes by slot 3, its norms issue at slot 4
                (DVE latency hidden behind B's steps, so the acc ring never
                stalls the unit after); B finishes at slot 7, norms at end.
                prev_mmajor: step m at slot m for both tiles instead — u0's
                PV must wait for late v tiles that are still DMA-paced.
                Pending transposes (from the unit before prev) drop into even
                slots, a full unit after their norms — the Ldweights that
                loads h_sb never reaches PE.SEQ before its data is ready.
                Fillers keep per-slot PE work above the ScalarE exp pace so
                the 2-deep s_ps ring never throttles the pipeline.
                """
                nh, t = u
                fillers = fillers or {}
                if u not in ust:
                    ust[u] = {"p": [], "h": [None] * 4, "left": 4}
                for m in range(len(ust[u]["p"]), MT):
                    s_slot(u, m)
                    if debug and u == (0, 0) and m == 0:
                        nc.sync.dma_start(dbg_p[:, :], ust[u]["p"][m][:])
                    if prev is not None:
                        if prev_mmajor:
                            pv_steps(prev, "A", [m])
                            pv_steps(prev, "B", [m])
                        elif m < 4:
                            pv_steps(prev, "A", [2 * m, 2 * m + 1])
                        else:
                            pv_steps(prev, "B", [2 * (m - 4), 2 * (m - 4) + 1])
                        if m == 4 and not prev_mmajor:
                            norm_tile(prev, "A")
                    if m % 2 == 0 and pending_T:
                        transpose_g(*pending_T.pop(0))
                    for fn in fillers.get(m, ()):
                        fn()
                if prev is not None:
                    if debug and prev == (0, 0):
                        for nm, d_d in (("accA", dbg_accA), ("accB", dbg_accB)):
                            stg = stage_pool.tile([128, 512], F32, name="dbgs",
                                                  tag="wps")
                            nc.vector.tensor_copy(stg[:], ust[prev][nm][:])
                            nc.sync.dma_start(d_d[:, :], stg[:])
                    if prev_mmajor:
                        norm_tile(prev, "A")
                    norm_tile(prev, "B")
                    if debug and prev == (0, 0):
                        for gg in range(4):
                            nc.sync.dma_start(dbg_h[gg], ust[prev]["h"][gg][:])
                    for gg in range(4):
                        pending_T.append((prev, gg))

            # ---- phase 3: half a y tile; one contiguous DMA per full tile
            # (half-tile stores cost ~1-2.5us each on SP.SEQ descriptor gen)
            y_tiles = {}

            def proj_half(m, ph, dve_evict=True, final=False):
                ps = ps_a.tile([128, 384], F32, name="ps_y", tag="mm1")
                for k in range(KT):
                    nc.tensor.matmul(ps[:], hT[k][:, m * 128:(m + 1) * 128],
                                     wp[k][:, ph * 384:(ph + 1) * 384],
                                     start=(k == 0), stop=(k == KT - 1))
                if ph == 0:
                    y_tiles[m] = y_pool.tile([128, C], F32, name="y_sb", tag="y")
                y_sb = y_tiles[m]
                if dve_evict:
                    nc.vector.tensor_copy(y_sb[:, ph * 384:(ph + 1) * 384], ps[:])
                else:
                    nc.scalar.copy(y_sb[:, ph * 384:(ph + 1) * 384], ps[:])
                if final:   # last tile: per-half stores shorten the exit path
                    nc.sync.dma_start(
                        y_d[m * 128:(m + 1) * 128, ph * 384:(ph + 1) * 384],
                        y_sb[:, ph * 384:(ph + 1) * 384])
                elif ph == 1:
                    nc.sync.dma_start(y_d[m * 128:(m + 1) * 128, :], y_sb[:])

            def load_wp():
                for k in range(KT):
                    stg = stage_pool.tile([128, C], F32R, name="wps", tag="wps")
                    nc.sync.dma_start(stg[:], wproj_d[k * 128:(k + 1) * 128, :])
                    nc.vector.tensor_copy(wp[k][:], stg[:])

            # ---- emission schedule ----
            # Unit order: (0,0), (1,0) — the second costs no new weight DMA
            # (q0h1 computes from the already-loaded t0 slices), keeping the
            # exp stream alive while v tiles cook — then nh0's remaining t
            # (so n-half-0 proj can overlap late nh1 units), then nh1's.
            # Filler placement keeps every s_phase slot's PE work at or above
            # the ScalarE exp pace (EDF for qk halves: kT full + qT's working
            # half before a pair's first unit, the other qT half before the
            # pair's nh1 unit; v tiles late enough for their DMA; proj last).
            units = ([(0, 0), (1, 0)] + [(0, t) for t in range(1, 6)]
                     + [(1, t) for t in range(1, 6)])
            qk_headgroup([(0, 0), (6, 0)])                   # xt-h0 paced
            qk_headgroup([(6, 1), (0, 1)])                   # xt-h1 paced
            s_phase(units[0], None, {
                4: [lambda: v_half(0, 0), lambda: v_half(0, 1)],
                5: [lambda: v_half(1, 0), lambda: v_half(1, 1)],
                6: [lambda: v_half(2, 0), lambda: v_half(2, 1)],
                7: [lambda: v_half(3, 0), lambda: v_half(3, 1)]})
            F = {}
            F[1] = {0: [lambda: v_half(4, 0), lambda: v_half(4, 1)],
                    1: [lambda: v_half(5, 0), lambda: v_half(5, 1)],
                    2: [lambda: v_half(6, 0), lambda: v_half(6, 1)],
                    3: [lambda: v_half(7, 0), lambda: v_half(7, 1)],
                    4: [lambda: qk_half(1, 0)], 5: [lambda: qk_half(7, 0)],
                    6: [lambda: qk_half(7, 1)]}
            for i, tq in ((2, 2), (3, 3), (4, 4), (5, 5)):
                F[i] = {0: [lambda tq=tq: qk_half(tq, 0)],
                        3: [lambda tq=tq: qk_half(tq + 6, 0)],
                        6: [lambda tq=tq: qk_half(tq + 6, 1)]}
            F[6] = {1: [lambda: qk_half(1, 1)], 3: [load_wp]}
            F[7] = {1: [lambda: qk_half(2, 1)]}
            for i in (8, 9, 10):
                mp = i - 8
                F[i] = {1: [lambda i=i: qk_half(i - 5, 1)],
                        3: [lambda mp=mp: proj_half(mp, 0)],
                        5: [lambda mp=mp: proj_half(mp, 1)]}
            F[11] = {1: [lambda: proj_half(3, 0)], 3: [lambda: proj_half(3, 1)]}
            for i in range(1, 12):
                s_phase(units[i], units[i - 1], F.get(i), prev_mmajor=(i == 1))
            # tail: last unit's PV / norms / transposes / proj, pipelined
            # (transposes run one g ahead of proj so the hT evict's DVE
            # latency hides under the previous proj's matmuls)
            last = units[11]
            for m in range(4):
                pv_steps(last, "A", [2 * m, 2 * m + 1])
                if pending_T:                    # u10's deferred transposes
                    transpose_g(*pending_T.pop(0))
            norm_tile(last, "A")
            for m in range(4):
                pv_steps(last, "B", [2 * m, 2 * m + 1])
                if pending_T:
                    transpose_g(*pending_T.pop(0))
            norm_tile(last, "B")
            transpose_g(last, 0)
            transpose_g(last, 1)
            for gg in range(4):
                if gg < 2:
                    transpose_g(last, gg + 2)
                proj_half(4 + gg, 0, dve_evict=False, final=(gg == 3))
                proj_half(4 + gg, 1, dve_evict=False, final=(gg == 3))
            if debug:
                for t in range(12):
                    nc.sync.dma_start(dbg_qkT[t], qkT[t][:])
                for m in range(MT):
                    nc.sync.dma_start(dbg_v[m], v_bf[m][:])
                for k in range(KT):
                    nc.sync.dma_start(dbg_hT[k], hT[k][:])

    nc.compile()
    return nc


def _run(inputs, trace=False, trace_kwargs=None):
    global _cached_nc
    x = np.asarray(inputs["x"], dtype=np.float32)
    wqkv = np.ascontiguousarray(np.asarray(inputs["W_qkv"], dtype=np.float32))
    wproj = np.ascontiguousarray(np.asarray(inputs["W_proj"], dtype=np.float32))
    xT = np.ascontiguousarray(x.transpose(0, 2, 1))  # [B, C, N]

    if _cached_nc is None:
        _cached_nc = build_program()
    nc = _cached_nc

    in_maps = [{"xT": xT[b], "wqkv": wqkv, "wproj": wproj} for b in range(B)]
    kwargs = {}
    if trace:
        kwargs["trace"] = True
        if trace_kwargs:
            kwargs.update(trace_kwargs)
    try:
        res = run_bass_kernel_spmd(nc, in_maps, core_ids=list(range(B)), **kwargs)
    except Exception:
        # transient axon/PJRT hiccups happen; one retry
        res = run_bass_kernel_spmd(nc, in_maps, core_ids=list(range(B)), **kwargs)
    out = np.stack([r["y"] for r in res.results], axis=0)
    return out, res


def kernel(**inputs):
    out, _ = _run(inputs)
    return out
